# revision 2
# baseline (speedup 1.0000x reference)
"""Trainium2 Bass kernel for causal multi-head attention with RoPE.

Problem: x[2,2048,2048] -> qkv proj -> RoPE(q,k) -> causal softmax attention
(16 heads, hd=128) -> out proj.  Sharding: tensor-parallel over heads
(2 heads/core x 8 cores); the output projection contraction is restored
with an AllToAll (head-shards -> sequence-shards), so each core computes
a disjoint 512-row slice of the final [4096, 2048] output.

All matmuls run as float32r (full-rate fp32 PE mode, ~1.6e-4 rel err on a
2048-deep contraction).  Softmax skips the max-subtraction (scores are
O(1) by construction) which makes the streaming k-tile loop trivial:
exp on ACT from PSUM, column sums accumulated on GPSIMD, partition-reduced
with the gpsimd partition_all_reduce ucode, one reciprocal, one normalize.
"""

import os
import sys

if "/opt/trn_rl_repo" not in sys.path:
    sys.path.insert(0, "/opt/trn_rl_repo")

import numpy as np

B, S, D = 2, 2048, 2048
H, HD = 16, 128
NCORES = 8
HPC = H // NCORES          # heads per core (2)
ROPE_BASE = 10000.0
SCALE = 1.0 / float(np.sqrt(HD))
SC = 256                   # QKV matmul free-dim chunk (s positions)
KSUB = D // 128            # 16 contraction subtiles

_CACHE = {}


def _install_trace_shim():
    """Optionally register the axon NTFF profile hook (for test.py tracing)."""
    try:
        import types

        if "antenv.axon_hooks" in sys.modules:
            return True
        import antenv
        from trn_agent_boot.trn_boot import _ntff_profile_via_ctypes

        hook = _ntff_profile_via_ctypes("/opt/axon/libaxon_pjrt.so")
        mod = types.ModuleType("antenv.axon_hooks")
        _state = {"hook": hook}
        mod.get_axon_ntff_profile_hook = lambda: _state["hook"]
        mod.set_axon_ntff_profile_hook = lambda h: _state.__setitem__("hook", h)
        sys.modules["antenv.axon_hooks"] = mod
        antenv.axon_hooks = mod
        return True
    except Exception:
        return False


def _build():
    import concourse.bass as bass  # noqa: F401
    import concourse.mybir as mybir
    import concourse.tile as tile
    from concourse import bacc
    from concourse import bass_isa
    from concourse.masks import make_identity

    f32 = mybir.dt.float32
    f32r = mybir.dt.float32r
    EXP = mybir.ActivationFunctionType.Exp

    nc = bacc.Bacc("TRN2", target_bir_lowering=False, debug=False,
                   num_devices=NCORES)

    xT = nc.dram_tensor("xT", [128, KSUB, B * S], f32r, kind="ExternalInput")
    wqkv = nc.dram_tensor("wqkv", [128, KSUB, 3 * HPC * HD], f32r,
                          kind="ExternalInput")
    wout = nc.dram_tensor("wout", [128, KSUB, D], f32r, kind="ExternalInput")
    cosg = nc.dram_tensor("cosg", [128, S], f32, kind="ExternalInput")
    sing = nc.dram_tensor("sing", [128, S], f32, kind="ExternalInput")
    mstr = nc.dram_tensor("mstr", [128, 896], f32, kind="ExternalInput")
    y = nc.dram_tensor("y", [B * S // NCORES, D], f32, kind="ExternalOutput")

    NM = 3 * HPC           # 6 qkv feature blocks of 128
    NQC = S // SC          # qkv s-chunks per batch
    NKT = S // 128         # 16 key tiles

    with tile.TileContext(nc) as tc:
        with tc.tile_pool(name="const", bufs=1) as cp, \
             tc.tile_pool(name="stage", bufs=2) as stp, \
             tc.tile_pool(name="dram", bufs=1, space="DRAM") as dp, \
             tc.tile_pool(name="psA", bufs=4, space="PSUM") as psA, \
             tc.tile_pool(name="psOut", bufs=1, space="PSUM") as psO:

            cos_sb = cp.tile([128, S], f32, name="cos_sb")
            sin_sb = cp.tile([128, S], f32, name="sin_sb")
            strip = cp.tile([128, 896], f32, name="strip")
            ident = cp.tile([128, 128], f32, name="ident")
            nc.sync.dma_start(cos_sb[:], cosg.ap())
            nc.sync.dma_start(sin_sb[:], sing.ap())
            nc.sync.dma_start(strip[:], mstr.ap())
            make_identity(nc, ident[:])

            ib = dp.tile([NCORES, HPC, 128, 512], f32r, name="ib")
            ob = dp.tile([NCORES, HPC, 128, 512], f32r, name="ob")

            with tc.tile_pool(name="w", bufs=1) as wp, \
                 tc.tile_pool(name="xc", bufs=2) as xp, \
                 tc.tile_pool(name="qkv", bufs=1) as qp, \
                 tc.tile_pool(name="attn", bufs=1) as ap_, \
                 tc.tile_pool(name="small", bufs=3) as ep:

                w_sb = wp.tile([128, KSUB, NM * 128], f32r, name="w_sb")
                nc.sync.dma_start(w_sb[:], wqkv.ap())

                for b in range(B):
                    # ---- QKV projection (feature-on-partition layout)
                    qkT = qp.tile([128, 2 * HPC, S], f32r, tag="qkT")
                    vT = qp.tile([128, HPC, S], f32, tag="vT")
                    for sc in range(NQC):
                        xc = xp.tile([128, KSUB, SC], f32r, tag="xc")
                        off = b * S + sc * SC
                        nc.sync.dma_start(xc[:], xT.ap()[:, :, off:off + SC])
                        for m in range(NM):
                            ps = psA.tile([128, 512], f32, tag="bank")
                            for k in range(KSUB):
                                nc.tensor.matmul(
                                    ps[:, :SC],
                                    w_sb[:, k, m * 128:(m + 1) * 128],
                                    xc[:, k],
                                    start=(k == 0), stop=(k == KSUB - 1))
                            dst = (qkT[:, m] if m < 2 * HPC
                                   else vT[:, m - 2 * HPC])
                            nc.vector.tensor_copy(
                                dst[:, sc * SC:(sc + 1) * SC], ps[:, :SC])

                    # ---- RoPE on q (m=0..HPC-1) and k (m=HPC..2*HPC-1)
                    for m in range(2 * HPC):
                        for c in range(S // 512):
                            sl = slice(c * 512, (c + 1) * 512)
                            rt = ep.tile([128, 512], f32, tag="rot")
                            nc.gpsimd.tensor_copy(
                                rt[0:64, :], qkT[64:128, m, sl].bitcast(f32))
                            nc.gpsimd.tensor_copy(
                                rt[64:128, :], qkT[0:64, m, sl].bitcast(f32))
                            nc.vector.tensor_mul(rt[:], rt[:], sin_sb[:, sl])
                            nc.vector.tensor_mul(qkT[:, m, sl], qkT[:, m, sl],
                                                 cos_sb[:, sl])
                            nc.vector.tensor_add(qkT[:, m, sl], qkT[:, m, sl],
                                                 rt[:])

                    # ---- attention per head
                    for h in range(HPC):
                        # V natural layout via PE transpose of vT tiles
                        Vh = ap_.tile([128, NKT, 128], f32r, tag="Vh")
                        for kt in range(NKT):
                            pt = psA.tile([128, 512], f32, tag="bank")
                            nc.tensor.transpose(
                                pt[:, :128],
                                vT[:, h, kt * 128:(kt + 1) * 128], ident[:])
                            nc.vector.tensor_copy(Vh[:, kt], pt[:, :128])

                        outT = psO.tile([128, S], f32, tag="outT")
                        acc = ap_.tile([128, S], f32, tag="acc")
                        acc2 = ap_.tile([128, S], f32, tag="acc2")
                        for kt in range(NKT):
                            q0 = 512 * (kt // 4)
                            for c in range((S - q0) // 512):
                                qs = q0 + c * 512
                                sp = psA.tile([128, 512], f32, tag="bank")
                                nc.tensor.matmul(
                                    sp[:],
                                    qkT[:, HPC + h, kt * 128:(kt + 1) * 128],
                                    qkT[:, h, qs:qs + 512],
                                    start=True, stop=True)
                                et = ep.tile([128, 512], f32r, tag="expT")
                                nc.scalar.activation(et[:], sp[:], EXP,
                                                     scale=SCALE)
                                if c == 0:
                                    moff = 384 - 128 * (kt % 4)
                                    nc.vector.tensor_mul(
                                        et[:], et[:],
                                        strip[:, moff:moff + 512])
                                if kt == 0:
                                    nc.gpsimd.tensor_copy(
                                        acc[:, qs:qs + 512], et[:].bitcast(f32))
                                else:
                                    nc.gpsimd.tensor_add(
                                        acc[:, qs:qs + 512],
                                        acc[:, qs:qs + 512],
                                        et[:].bitcast(f32))
                                nc.tensor.matmul(
                                    outT[:, qs:qs + 512],
                                    Vh[:, kt],
                                    et[:],
                                    start=(kt == 0),
                                    stop=(kt == 4 * (qs // 512) + 3))

                        nc.gpsimd.partition_all_reduce(
                            acc2[:], acc[:], 128, bass_isa.ReduceOp.add)
                        nc.vector.reciprocal(acc[:], acc2[:])
                        st = stp.tile([128, S], f32r, tag="st")
                        nc.vector.tensor_mul(st[:], outT[:], acc[:])
                        for jj in range(4):
                            nc.sync.dma_start(ib[4 * b + jj, h],
                                              st[:, jj * 512:(jj + 1) * 512])

            # ---- AllToAll: head-shards -> sequence-shards
            with tc.tile_pool(name="proj", bufs=2) as pp:
                nc.gpsimd.collective_compute(
                    "AllToAll", mybir.AluOpType.bypass,
                    replica_groups=[list(range(NCORES))],
                    ins=[ib.opt()], outs=[ob.opt()])

                lhs_sb = pp.tile([128, KSUB, 512], f32r, tag="lhs")
                nc.sync.dma_start(
                    lhs_sb[:], ob[:].rearrange("i hh p s -> p (i hh) s"))
                for n in range(4):
                    wo = pp.tile([128, KSUB, 512], f32r, tag="wo")
                    nc.sync.dma_start(wo[:], wout.ap()[:, :, n * 512:(n + 1) * 512])
                    for m in range(4):
                        ps = psA.tile([128, 512], f32, tag="bank")
                        for k in range(KSUB):
                            nc.tensor.matmul(
                                ps[:],
                                lhs_sb[:, k, m * 128:(m + 1) * 128],
                                wo[:, k],
                                start=(k == 0), stop=(k == KSUB - 1))
                        ys = stp.tile([128, 512], f32, tag="ys")
                        nc.vector.tensor_copy(ys[:], ps[:])
                        nc.sync.dma_start(
                            y.ap()[m * 128:(m + 1) * 128, n * 512:(n + 1) * 512],
                            ys[:])

    nc.finalize()
    return nc


def _host_inputs(x, w_qkv, w_out):
    xTr = np.ascontiguousarray(
        x.reshape(B * S, D).T.reshape(KSUB, 128, B * S).transpose(1, 0, 2))
    woutr = np.ascontiguousarray(
        w_out.reshape(KSUB, 128, D).transpose(1, 0, 2))

    half = HD // 2
    inv = (1.0 / (ROPE_BASE ** (np.arange(half, dtype=np.float32) / half))
           ).astype(np.float32)
    ang = (np.arange(S, dtype=np.float32)[:, None] * inv[None, :])  # [S, 64]
    c = np.cos(ang).astype(np.float32).T      # [64, S]
    s = np.sin(ang).astype(np.float32).T
    cosg = np.ascontiguousarray(np.concatenate([c, c], axis=0))
    sing = np.ascontiguousarray(np.concatenate([-s, s], axis=0))

    u = np.arange(896)[None, :] - 384
    p = np.arange(128)[:, None]
    mstr = (u >= p).astype(np.float32)

    maps = []
    for i in range(NCORES):
        h0, h1 = 2 * i, 2 * i + 1
        blocks = []
        for base in (0, D, 2 * D):
            blocks.append(w_qkv[:, base + 128 * h0:base + 128 * (h0 + 1)])
            blocks.append(w_qkv[:, base + 128 * h1:base + 128 * (h1 + 1)])
        shard = np.concatenate(blocks, axis=1)  # [D, 768]
        shard = np.ascontiguousarray(
            shard.reshape(KSUB, 128, 3 * HPC * HD).transpose(1, 0, 2))
        maps.append({"xT": xTr, "wqkv": shard, "wout": woutr,
                     "cosg": cosg, "sing": sing, "mstr": mstr})
    return maps


def kernel(x, w_qkv, w_out):
    from concourse.bass_utils import run_bass_kernel_spmd

    x = np.asarray(x, dtype=np.float32)
    w_qkv = np.asarray(w_qkv, dtype=np.float32)
    w_out = np.asarray(w_out, dtype=np.float32)

    if "nc" not in _CACHE:
        _CACHE["nc"] = _build()
    nc = _CACHE["nc"]

    trace = bool(int(os.environ.get("KERNEL_TRACE", "0")))
    if trace:
        trace = _install_trace_shim()

    in_maps = _host_inputs(x, w_qkv, w_out)
    res = run_bass_kernel_spmd(nc, in_maps, core_ids=list(range(NCORES)),
                               trace=trace)
    _CACHE["last_result"] = res
    out = np.concatenate([res.results[i]["y"] for i in range(NCORES)], axis=0)
    return out.reshape(B, S, D)


# revision 5
# speedup vs baseline: 1.3126x; 1.3126x over previous
"""Trainium2 Bass kernel for causal multi-head attention with RoPE.

Problem: x[2,2048,2048] -> qkv proj -> RoPE(q,k) -> causal softmax attention
(16 heads, hd=128) -> out proj.  Sharding: tensor-parallel over heads
(2 heads/core x 8 cores); the output projection contraction is restored
with an AllToAll (head-shards -> sequence-shards), so each core computes
a disjoint 512-row slice of the final [4096, 2048] output.

All matmuls run as float32r (full-rate fp32 PE mode, ~1.6e-4 rel err on a
2048-deep contraction).  Softmax skips the max-subtraction (scores are
O(1) by construction); the causal mask is applied inside PSUM by
accumulating a -1e9 constant via a PE identity-matmul, so exp() sees
-inf-like scores and the whole mask costs ~213ns/k-tile of PE time.
"""

import os
import sys

if "/opt/trn_rl_repo" not in sys.path:
    sys.path.insert(0, "/opt/trn_rl_repo")

import numpy as np

B, S, D = 2, 2048, 2048
H, HD = 16, 128
NCORES = 8
HPC = H // NCORES          # heads per core (2)
ROPE_BASE = 10000.0
SCALE = 1.0 / float(np.sqrt(HD))
SC = 256                   # QKV matmul free-dim chunk (s positions)
KSUB = D // 128            # 16 contraction subtiles

_CACHE = {}


def _install_trace_shim():
    """Optionally register the axon NTFF profile hook (for test.py tracing)."""
    try:
        import types

        if "antenv.axon_hooks" in sys.modules:
            return True
        import antenv
        from trn_agent_boot.trn_boot import _ntff_profile_via_ctypes

        hook = _ntff_profile_via_ctypes("/opt/axon/libaxon_pjrt.so")
        mod = types.ModuleType("antenv.axon_hooks")
        _state = {"hook": hook}
        mod.get_axon_ntff_profile_hook = lambda: _state["hook"]
        mod.set_axon_ntff_profile_hook = lambda h: _state.__setitem__("hook", h)
        sys.modules["antenv.axon_hooks"] = mod
        antenv.axon_hooks = mod
        return True
    except Exception:
        return False


def _build():
    import concourse.bass as bass  # noqa: F401
    import concourse.mybir as mybir
    import concourse.tile as tile
    from concourse import bacc
    from concourse import bass_isa
    from concourse.masks import make_identity

    f32 = mybir.dt.float32
    f32r = mybir.dt.float32r
    EXP = mybir.ActivationFunctionType.Exp

    nc = bacc.Bacc("TRN2", target_bir_lowering=False, debug=False,
                   num_devices=NCORES)

    xT = nc.dram_tensor("xT", [128, KSUB, B * S], f32r, kind="ExternalInput")
    wqkv = nc.dram_tensor("wqkv", [128, KSUB, 3 * HPC * HD], f32r,
                          kind="ExternalInput")
    wout = nc.dram_tensor("wout", [128, KSUB, D], f32r, kind="ExternalInput")
    cosg = nc.dram_tensor("cosg", [128, S], f32, kind="ExternalInput")
    sing = nc.dram_tensor("sing", [128, S], f32, kind="ExternalInput")
    mneg = nc.dram_tensor("mneg", [128, 4, 512], f32r, kind="ExternalInput")
    y = nc.dram_tensor("y", [B * S // NCORES, D], f32, kind="ExternalOutput")

    NQC = S // SC          # qkv s-chunks per batch
    NKT = S // 128         # 16 key tiles
    VOFF = 2 * HPC * HD    # v block column offset in w_sb (512)

    with tile.TileContext(nc) as tc:
        with tc.tile_pool(name="const", bufs=1) as cp, \
             tc.tile_pool(name="stage", bufs=2) as stp, \
             tc.tile_pool(name="dram", bufs=1, space="DRAM") as dp, \
             tc.tile_pool(name="psA", bufs=4, space="PSUM") as psA, \
             tc.tile_pool(name="psOut", bufs=1, space="PSUM") as psO:

            cos_sb = cp.tile([128, S], f32, name="cos_sb")
            sin_sb = cp.tile([128, S], f32, name="sin_sb")
            mneg_sb = cp.tile([128, 4, 512], f32r, name="mneg_sb")
            ident = cp.tile([128, 128], f32, name="ident")
            identR = cp.tile([128, 128], f32r, name="identR")
            nc.sync.dma_start(cos_sb[:], cosg.ap())
            nc.sync.dma_start(sin_sb[:], sing.ap())
            nc.sync.dma_start(mneg_sb[:], mneg.ap())
            make_identity(nc, ident[:])
            nc.vector.tensor_copy(identR[:], ident[:])

            ib = dp.tile([NCORES, HPC, 128, 512], f32r, name="ib")
            ob = dp.tile([NCORES, HPC, 128, 512], f32r, name="ob")

            with tc.tile_pool(name="w", bufs=1) as wp, \
                 tc.tile_pool(name="xc", bufs=2) as xp, \
                 tc.tile_pool(name="qkv", bufs=1) as qp, \
                 tc.tile_pool(name="attn", bufs=1) as ap_, \
                 tc.tile_pool(name="rotp", bufs=1) as rp, \
                 tc.tile_pool(name="small", bufs=4) as ep:

                w_sb = wp.tile([128, KSUB, 3 * HPC * HD], f32r, name="w_sb")
                nc.sync.dma_start(w_sb[:], wqkv.ap())

                for b in range(B):
                    # ---- QKV projection.
                    # q,k land feature-on-partition (qkT); v lands in natural
                    # [keys, hd] layout directly (xT slab as the stationary
                    # operand), which is exactly what the AV matmul wants.
                    qkT = qp.tile([128, 2 * HPC, S], f32r, tag="qkT")
                    Vn = qp.tile([128, NKT, HPC * HD], f32r, tag="Vn")
                    for sc in range(NQC):
                        xc = xp.tile([128, KSUB, SC], f32r, tag="xc")
                        off = b * S + sc * SC
                        nc.sync.dma_start(xc[:], xT.ap()[:, :, off:off + SC])
                        for m in range(2 * HPC):
                            ps = psA.tile([128, 512], f32, tag="bank")
                            for k in range(KSUB):
                                nc.tensor.matmul(
                                    ps[:, :SC],
                                    w_sb[:, k, m * 128:(m + 1) * 128],
                                    xc[:, k],
                                    start=(k == 0), stop=(k == KSUB - 1))
                            nc.vector.tensor_copy(
                                qkT[:, m, sc * SC:(sc + 1) * SC], ps[:, :SC])
                        for st2 in range(SC // 128):
                            ps = psA.tile([128, 512], f32, tag="bank")
                            for k in range(KSUB):
                                nc.tensor.matmul(
                                    ps[:, :HPC * HD],
                                    xc[:, k, st2 * 128:(st2 + 1) * 128],
                                    w_sb[:, k, VOFF:VOFF + HPC * HD],
                                    start=(k == 0), stop=(k == KSUB - 1))
                            nc.vector.tensor_copy(
                                Vn[:, sc * (SC // 128) + st2],
                                ps[:, :HPC * HD])

                    # ---- RoPE on q (m<HPC) and k (m>=HPC), fused halves:
                    # rt[0:64]   = t[64:128] * (-sin)
                    # rt[64:128] = t[0:64]   * (+sin)
                    # t *= cos ; t += rt
                    for m in range(2 * HPC):
                        rt = rp.tile([128, S], f32, tag="rot")
                        nc.vector.tensor_mul(rt[0:64, :],
                                             qkT[64:128, m].bitcast(f32),
                                             sin_sb[64:128, :])
                        nc.vector.tensor_mul(rt[64:128, :],
                                             qkT[0:64, m].bitcast(f32),
                                             sin_sb[0:64, :])
                        nc.vector.tensor_mul(qkT[:, m], qkT[:, m], cos_sb[:])
                        nc.vector.tensor_add(qkT[:, m], qkT[:, m], rt[:])

                    # ---- attention per head
                    for h in range(HPC):
                        outT = psO.tile([128, S], f32, tag="outT")
                        acc = ap_.tile([128, S], f32, tag="acc")
                        acc2 = ap_.tile([128, S], f32, tag="acc2")
                        for kt in range(NKT):
                            q0 = 512 * (kt // 4)
                            nch = (S - q0) // 512
                            sps = []
                            # scores^T for this k-tile (grouped LDWEIGHTS):
                            # the diagonal chunk gets the causal -1e9 mask
                            # accumulated first via an identity matmul.
                            for c in range(nch):
                                qs = q0 + c * 512
                                sp = psA.tile([128, 512], f32, tag="bank")
                                sps.append(sp)
                                if c == 0:
                                    nc.tensor.matmul(sp[:], identR[:],
                                                     mneg_sb[:, kt % 4],
                                                     start=True, stop=False)
                            for c in range(nch):
                                qs = q0 + c * 512
                                nc.tensor.matmul(
                                    sps[c][:],
                                    qkT[:, HPC + h, kt * 128:(kt + 1) * 128],
                                    qkT[:, h, qs:qs + 512],
                                    start=(c != 0), stop=True)
                            ets = []
                            for c in range(nch):
                                et = ep.tile([128, 512], f32r, tag="expT")
                                ets.append(et)
                                nc.scalar.activation(et[:], sps[c][:], EXP,
                                                     scale=SCALE)
                            for c in range(nch):
                                qs = q0 + c * 512
                                eng = nc.gpsimd if (kt + c) % 2 else nc.vector
                                if kt == 0:
                                    eng.tensor_copy(acc[:, qs:qs + 512],
                                                    ets[c][:].bitcast(f32))
                                else:
                                    eng.tensor_add(acc[:, qs:qs + 512],
                                                   acc[:, qs:qs + 512],
                                                   ets[c][:].bitcast(f32))
                            for c in range(nch):
                                qs = q0 + c * 512
                                nc.tensor.matmul(
                                    outT[:, qs:qs + 512],
                                    Vn[:, kt, h * 128:(h + 1) * 128],
                                    ets[c][:],
                                    start=(kt == 0),
                                    stop=(kt == 4 * (qs // 512) + 3))

                        nc.gpsimd.partition_all_reduce(
                            acc2[:], acc[:], 128, bass_isa.ReduceOp.add)
                        nc.vector.reciprocal_approx_fast(acc[:], acc2[:])
                        st = stp.tile([128, S], f32r, tag="st")
                        nc.vector.tensor_mul(st[:], outT[:], acc[:])
                        for jj in range(4):
                            nc.sync.dma_start(ib[4 * b + jj, h],
                                              st[:, jj * 512:(jj + 1) * 512])

            # ---- AllToAll: head-shards -> sequence-shards
            with tc.tile_pool(name="proj", bufs=2) as pp:
                nc.gpsimd.collective_compute(
                    "AllToAll", mybir.AluOpType.bypass,
                    replica_groups=[list(range(NCORES))],
                    ins=[ib.opt()], outs=[ob.opt()])

                lhs_sb = pp.tile([128, KSUB, 512], f32r, tag="lhs")
                nc.sync.dma_start(
                    lhs_sb[:], ob[:].rearrange("i hh p s -> p (i hh) s"))
                for n in range(4):
                    wo = pp.tile([128, KSUB, 512], f32r, tag="wo")
                    nc.sync.dma_start(wo[:], wout.ap()[:, :, n * 512:(n + 1) * 512])
                    for m in range(4):
                        ps = psA.tile([128, 512], f32, tag="bank")
                        for k in range(KSUB):
                            nc.tensor.matmul(
                                ps[:],
                                lhs_sb[:, k, m * 128:(m + 1) * 128],
                                wo[:, k],
                                start=(k == 0), stop=(k == KSUB - 1))
                        ys = stp.tile([128, 512], f32, tag="ys")
                        nc.vector.tensor_copy(ys[:], ps[:])
                        nc.sync.dma_start(
                            y.ap()[m * 128:(m + 1) * 128, n * 512:(n + 1) * 512],
                            ys[:])

    nc.finalize()
    return nc


def _host_inputs(x, w_qkv, w_out):
    xTr = np.ascontiguousarray(
        x.reshape(B * S, D).T.reshape(KSUB, 128, B * S).transpose(1, 0, 2))
    woutr = np.ascontiguousarray(
        w_out.reshape(KSUB, 128, D).transpose(1, 0, 2))

    half = HD // 2
    inv = (1.0 / (ROPE_BASE ** (np.arange(half, dtype=np.float32) / half))
           ).astype(np.float32)
    ang = (np.arange(S, dtype=np.float32)[:, None] * inv[None, :])  # [S, 64]
    c = np.cos(ang).astype(np.float32).T      # [64, S]
    s = np.sin(ang).astype(np.float32).T
    cosg = np.ascontiguousarray(np.concatenate([c, c], axis=0))
    sing = np.ascontiguousarray(np.concatenate([s, -s], axis=0))

    # mneg[p, m, j] = -1e9 where key-row p masks query-col j in the
    # diagonal 512-chunk of k-tile with kt%4 == m, else 0.
    mm = np.arange(4)[None, :, None]
    j = np.arange(512)[None, None, :]
    p = np.arange(128)[:, None, None]
    mneg = np.where(j >= 128 * mm + p, 0.0, -1e9).astype(np.float32)

    maps = []
    for i in range(NCORES):
        h0, h1 = 2 * i, 2 * i + 1
        blocks = []
        for base in (0, D, 2 * D):
            blocks.append(w_qkv[:, base + 128 * h0:base + 128 * (h0 + 1)])
            blocks.append(w_qkv[:, base + 128 * h1:base + 128 * (h1 + 1)])
        shard = np.concatenate(blocks, axis=1)  # [D, 768]
        shard = np.ascontiguousarray(
            shard.reshape(KSUB, 128, 3 * HPC * HD).transpose(1, 0, 2))
        maps.append({"xT": xTr, "wqkv": shard, "wout": woutr,
                     "cosg": cosg, "sing": sing, "mneg": mneg})
    return maps


def kernel(x, w_qkv, w_out):
    from concourse.bass_utils import run_bass_kernel_spmd

    x = np.asarray(x, dtype=np.float32)
    w_qkv = np.asarray(w_qkv, dtype=np.float32)
    w_out = np.asarray(w_out, dtype=np.float32)

    if "nc" not in _CACHE:
        _CACHE["nc"] = _build()
    nc = _CACHE["nc"]

    trace = bool(int(os.environ.get("KERNEL_TRACE", "0")))
    if trace:
        trace = _install_trace_shim()

    in_maps = _host_inputs(x, w_qkv, w_out)
    res = run_bass_kernel_spmd(nc, in_maps, core_ids=list(range(NCORES)),
                               trace=trace)
    _CACHE["last_result"] = res
    out = np.concatenate([res.results[i]["y"] for i in range(NCORES)], axis=0)
    return out.reshape(B, S, D)


# revision 6
# speedup vs baseline: 1.3273x; 1.0113x over previous
"""Trainium2 Bass kernel for causal multi-head attention with RoPE.

Problem: x[2,2048,2048] -> qkv proj -> RoPE(q,k) -> causal softmax attention
(16 heads, hd=128) -> out proj.  Sharding: tensor-parallel over heads
(2 heads/core x 8 cores); the output projection contraction is restored
with an AllToAll (head-shards -> sequence-shards), so each core computes
a disjoint 512-row slice of the final [4096, 2048] output.

All matmuls run as float32r (full-rate fp32 PE mode, ~1.6e-4 rel err on a
2048-deep contraction).  Softmax skips the max-subtraction (scores are
O(1) by construction); the causal mask is applied inside PSUM by
accumulating a -1e9 constant via a PE identity-matmul, so exp() sees
-inf-like scores and the whole mask costs ~213ns/k-tile of PE time.
"""

import os
import sys

if "/opt/trn_rl_repo" not in sys.path:
    sys.path.insert(0, "/opt/trn_rl_repo")

import numpy as np

B, S, D = 2, 2048, 2048
H, HD = 16, 128
NCORES = 8
HPC = H // NCORES          # heads per core (2)
ROPE_BASE = 10000.0
SCALE = 1.0 / float(np.sqrt(HD))
SC = 512                   # QKV matmul free-dim chunk (s positions)
KSUB = D // 128            # 16 contraction subtiles

_CACHE = {}


def _install_trace_shim():
    """Optionally register the axon NTFF profile hook (for test.py tracing)."""
    try:
        import types

        if "antenv.axon_hooks" in sys.modules:
            return True
        import antenv
        from trn_agent_boot.trn_boot import _ntff_profile_via_ctypes

        hook = _ntff_profile_via_ctypes("/opt/axon/libaxon_pjrt.so")
        mod = types.ModuleType("antenv.axon_hooks")
        _state = {"hook": hook}
        mod.get_axon_ntff_profile_hook = lambda: _state["hook"]
        mod.set_axon_ntff_profile_hook = lambda h: _state.__setitem__("hook", h)
        sys.modules["antenv.axon_hooks"] = mod
        antenv.axon_hooks = mod
        return True
    except Exception:
        return False


def _build():
    import concourse.bass as bass  # noqa: F401
    import concourse.mybir as mybir
    import concourse.tile as tile
    from concourse import bacc
    from concourse import bass_isa
    from concourse.masks import make_identity

    f32 = mybir.dt.float32
    f32r = mybir.dt.float32r
    EXP = mybir.ActivationFunctionType.Exp

    nc = bacc.Bacc("TRN2", target_bir_lowering=False, debug=False,
                   num_devices=NCORES)

    xT = nc.dram_tensor("xT", [128, KSUB, B * S], f32r, kind="ExternalInput")
    wqkv = nc.dram_tensor("wqkv", [128, KSUB, 3 * HPC * HD], f32r,
                          kind="ExternalInput")
    wout = nc.dram_tensor("wout", [128, KSUB, D], f32r, kind="ExternalInput")
    cosg = nc.dram_tensor("cosg", [128, S], f32, kind="ExternalInput")
    sing = nc.dram_tensor("sing", [128, S], f32, kind="ExternalInput")
    mneg = nc.dram_tensor("mneg", [128, 896], f32r, kind="ExternalInput")
    y = nc.dram_tensor("y", [B * S // NCORES, D], f32, kind="ExternalOutput")

    NQC = S // SC          # qkv s-chunks per batch
    NKT = S // 128         # 16 key tiles
    VOFF = 2 * HPC * HD    # v block column offset in w_sb (512)

    with tile.TileContext(nc) as tc:
        with tc.tile_pool(name="const", bufs=1) as cp, \
             tc.tile_pool(name="stage", bufs=2) as stp, \
             tc.tile_pool(name="dram", bufs=1, space="DRAM") as dp, \
             tc.tile_pool(name="psA", bufs=4, space="PSUM") as psA, \
             tc.tile_pool(name="psOut", bufs=1, space="PSUM") as psO:

            cos_sb = cp.tile([128, S], f32, name="cos_sb")
            sin_sb = cp.tile([128, S], f32, name="sin_sb")
            mneg_sb = cp.tile([128, 896], f32r, name="mneg_sb")
            ident = cp.tile([128, 128], f32, name="ident")
            identR = cp.tile([128, 128], f32r, name="identR")
            nc.sync.dma_start(cos_sb[:], cosg.ap())
            nc.sync.dma_start(sin_sb[:], sing.ap())
            nc.sync.dma_start(mneg_sb[:], mneg.ap())
            make_identity(nc, ident[:])
            nc.vector.tensor_copy(identR[:], ident[:])

            ib = dp.tile([NCORES, HPC, 128, 512], f32r, name="ib")
            ob = dp.tile([NCORES, HPC, 128, 512], f32r, name="ob")

            with tc.tile_pool(name="w", bufs=1) as wp, \
                 tc.tile_pool(name="xc", bufs=2) as xp, \
                 tc.tile_pool(name="qkv", bufs=1) as qp, \
                 tc.tile_pool(name="attn", bufs=1) as ap_, \
                 tc.tile_pool(name="rotp", bufs=1) as rp, \
                 tc.tile_pool(name="small", bufs=3) as ep:

                w_sb = wp.tile([128, KSUB, 3 * HPC * HD], f32r, name="w_sb")
                nc.sync.dma_start(w_sb[:], wqkv.ap())

                for b in range(B):
                    # ---- QKV projection.
                    # q,k land feature-on-partition (qkT); v lands in natural
                    # [keys, hd] layout directly (xT slab as the stationary
                    # operand), which is exactly what the AV matmul wants.
                    qkT = qp.tile([128, 2 * HPC, S], f32r, tag="qkT")
                    Vn = qp.tile([128, NKT, HPC * HD], f32r, tag="Vn")
                    for sc in range(NQC):
                        xc = xp.tile([128, KSUB, SC], f32r, tag="xc")
                        off = b * S + sc * SC
                        nc.sync.dma_start(xc[:], xT.ap()[:, :, off:off + SC])
                        for m in range(2 * HPC):
                            ps = psA.tile([128, 512], f32, tag="bank")
                            for k in range(KSUB):
                                nc.tensor.matmul(
                                    ps[:, :SC],
                                    w_sb[:, k, m * 128:(m + 1) * 128],
                                    xc[:, k],
                                    start=(k == 0), stop=(k == KSUB - 1))
                            nc.vector.tensor_copy(
                                qkT[:, m, sc * SC:(sc + 1) * SC], ps[:, :SC])
                        for st2 in range(SC // 128):
                            ps = psA.tile([128, 512], f32, tag="bank")
                            for k in range(KSUB):
                                nc.tensor.matmul(
                                    ps[:, :HPC * HD],
                                    xc[:, k, st2 * 128:(st2 + 1) * 128],
                                    w_sb[:, k, VOFF:VOFF + HPC * HD],
                                    start=(k == 0), stop=(k == KSUB - 1))
                            nc.vector.tensor_copy(
                                Vn[:, sc * (SC // 128) + st2],
                                ps[:, :HPC * HD])

                    # ---- RoPE on q (m<HPC) and k (m>=HPC), fused halves:
                    # rt[0:64]   = t[64:128] * (-sin)
                    # rt[64:128] = t[0:64]   * (+sin)
                    # t *= cos ; t += rt
                    for m in range(2 * HPC):
                        rt = rp.tile([128, S], f32, tag="rot", name="rt")
                        nc.vector.tensor_mul(rt[0:64, :],
                                             qkT[64:128, m].bitcast(f32),
                                             sin_sb[64:128, :])
                        nc.vector.tensor_mul(rt[64:128, :],
                                             qkT[0:64, m].bitcast(f32),
                                             sin_sb[0:64, :])
                        nc.vector.tensor_mul(qkT[:, m], qkT[:, m], cos_sb[:])
                        nc.vector.tensor_add(qkT[:, m], qkT[:, m], rt[:])

                    # ---- attention per head
                    for h in range(HPC):
                        outT = psO.tile([128, S], f32, tag="outT")
                        acc = ap_.tile([128, S], f32, tag="acc")
                        acc2 = rp.tile([128, S], f32, tag="rot", name="acc2")
                        for kt in range(NKT):
                            q0 = 512 * (kt // 4)
                            nch = (S - q0) // 512
                            sps = []
                            # scores^T for this k-tile (grouped LDWEIGHTS):
                            # the diagonal chunk gets the causal -1e9 mask
                            # accumulated first via an identity matmul.
                            for c in range(nch):
                                qs = q0 + c * 512
                                sp = psA.tile([128, 512], f32, tag="bank")
                                sps.append(sp)
                                if c == 0:
                                    moff = 384 - 128 * (kt % 4)
                                    nc.tensor.matmul(sp[:], identR[:],
                                                     mneg_sb[:, moff:moff + 512],
                                                     start=True, stop=False)
                            for c in range(nch):
                                qs = q0 + c * 512
                                nc.tensor.matmul(
                                    sps[c][:],
                                    qkT[:, HPC + h, kt * 128:(kt + 1) * 128],
                                    qkT[:, h, qs:qs + 512],
                                    start=(c != 0), stop=True)
                            ets = []
                            for c in range(nch):
                                et = ep.tile([128, 512], f32r, tag="expT")
                                ets.append(et)
                                nc.scalar.activation(et[:], sps[c][:], EXP,
                                                     scale=SCALE)
                            for c in range(nch):
                                qs = q0 + c * 512
                                eng = nc.gpsimd if (kt + c) % 2 else nc.vector
                                if kt == 0:
                                    eng.tensor_copy(acc[:, qs:qs + 512],
                                                    ets[c][:].bitcast(f32))
                                else:
                                    eng.tensor_add(acc[:, qs:qs + 512],
                                                   acc[:, qs:qs + 512],
                                                   ets[c][:].bitcast(f32))
                            for c in range(nch):
                                qs = q0 + c * 512
                                nc.tensor.matmul(
                                    outT[:, qs:qs + 512],
                                    Vn[:, kt, h * 128:(h + 1) * 128],
                                    ets[c][:],
                                    start=(kt == 0),
                                    stop=(kt == 4 * (qs // 512) + 3))

                        nc.gpsimd.partition_all_reduce(
                            acc2[:], acc[:], 128, bass_isa.ReduceOp.add)
                        nc.vector.reciprocal_approx_fast(acc[:], acc2[:])
                        st = rp.tile([128, S], f32r, tag="rot", name="st")
                        nc.vector.tensor_mul(st[:], outT[:], acc[:])
                        for jj in range(4):
                            nc.sync.dma_start(ib[4 * b + jj, h],
                                              st[:, jj * 512:(jj + 1) * 512])

            # ---- AllToAll: head-shards -> sequence-shards
            with tc.tile_pool(name="proj", bufs=2) as pp:
                nc.gpsimd.collective_compute(
                    "AllToAll", mybir.AluOpType.bypass,
                    replica_groups=[list(range(NCORES))],
                    ins=[ib.opt()], outs=[ob.opt()])

                lhs_sb = pp.tile([128, KSUB, 512], f32r, tag="lhs")
                nc.sync.dma_start(
                    lhs_sb[:], ob[:].rearrange("i hh p s -> p (i hh) s"))
                for n in range(4):
                    wo = pp.tile([128, KSUB, 512], f32r, tag="wo")
                    nc.sync.dma_start(wo[:], wout.ap()[:, :, n * 512:(n + 1) * 512])
                    for m in range(4):
                        ps = psA.tile([128, 512], f32, tag="bank")
                        for k in range(KSUB):
                            nc.tensor.matmul(
                                ps[:],
                                lhs_sb[:, k, m * 128:(m + 1) * 128],
                                wo[:, k],
                                start=(k == 0), stop=(k == KSUB - 1))
                        ys = stp.tile([128, 512], f32, tag="ys")
                        nc.vector.tensor_copy(ys[:], ps[:])
                        nc.sync.dma_start(
                            y.ap()[m * 128:(m + 1) * 128, n * 512:(n + 1) * 512],
                            ys[:])

    nc.finalize()
    return nc


def _host_inputs(x, w_qkv, w_out):
    xTr = np.ascontiguousarray(
        x.reshape(B * S, D).T.reshape(KSUB, 128, B * S).transpose(1, 0, 2))
    woutr = np.ascontiguousarray(
        w_out.reshape(KSUB, 128, D).transpose(1, 0, 2))

    half = HD // 2
    inv = (1.0 / (ROPE_BASE ** (np.arange(half, dtype=np.float32) / half))
           ).astype(np.float32)
    ang = (np.arange(S, dtype=np.float32)[:, None] * inv[None, :])  # [S, 64]
    c = np.cos(ang).astype(np.float32).T      # [64, S]
    s = np.sin(ang).astype(np.float32).T
    cosg = np.ascontiguousarray(np.concatenate([c, c], axis=0))
    sing = np.ascontiguousarray(np.concatenate([s, -s], axis=0))

    # mneg strip: mneg[p, u] = 0 where (u-384) >= p else -1e9; the diagonal
    # mask for k-tile kt is the 512-wide slice at offset 384-128*(kt%4).
    u = np.arange(896)[None, :] - 384
    p = np.arange(128)[:, None]
    mneg = np.where(u >= p, 0.0, -1e9).astype(np.float32)

    maps = []
    for i in range(NCORES):
        h0, h1 = 2 * i, 2 * i + 1
        blocks = []
        for base in (0, D, 2 * D):
            blocks.append(w_qkv[:, base + 128 * h0:base + 128 * (h0 + 1)])
            blocks.append(w_qkv[:, base + 128 * h1:base + 128 * (h1 + 1)])
        shard = np.concatenate(blocks, axis=1)  # [D, 768]
        shard = np.ascontiguousarray(
            shard.reshape(KSUB, 128, 3 * HPC * HD).transpose(1, 0, 2))
        maps.append({"xT": xTr, "wqkv": shard, "wout": woutr,
                     "cosg": cosg, "sing": sing, "mneg": mneg})
    return maps


def kernel(x, w_qkv, w_out):
    from concourse.bass_utils import run_bass_kernel_spmd

    x = np.asarray(x, dtype=np.float32)
    w_qkv = np.asarray(w_qkv, dtype=np.float32)
    w_out = np.asarray(w_out, dtype=np.float32)

    if "nc" not in _CACHE:
        _CACHE["nc"] = _build()
    nc = _CACHE["nc"]

    trace = bool(int(os.environ.get("KERNEL_TRACE", "0")))
    if trace:
        trace = _install_trace_shim()

    in_maps = _host_inputs(x, w_qkv, w_out)
    res = run_bass_kernel_spmd(nc, in_maps, core_ids=list(range(NCORES)),
                               trace=trace)
    _CACHE["last_result"] = res
    out = np.concatenate([res.results[i]["y"] for i in range(NCORES)], axis=0)
    return out.reshape(B, S, D)


# revision 12
# speedup vs baseline: 1.3891x; 1.0465x over previous
"""Trainium2 Bass kernel for causal multi-head attention with RoPE.

Problem: x[2,2048,2048] -> qkv proj -> RoPE(q,k) -> causal softmax attention
(16 heads, hd=128) -> out proj.  Sharding: tensor-parallel over heads
(2 heads/core x 8 cores); the output projection contraction is restored
with one AllToAll per batch (head-shards -> sequence-shards), overlapped
with the other batch's compute, so each core computes a disjoint
[2, 256, 2048] slice of the final output.

All matmuls run as float32r (full-rate fp32 PE mode, ~1.6e-4 rel err on a
2048-deep contraction).  Softmax skips the max-subtraction (scores are
O(1) by construction); the causal mask is accumulated into PSUM as a
-1e9 constant via a PE identity-matmul; softmax denominators are
partition-reduced and broadcast back with tiny ones-matmuls on the PE.
"""

import os
import sys

if "/opt/trn_rl_repo" not in sys.path:
    sys.path.insert(0, "/opt/trn_rl_repo")

import numpy as np

B, S, D = 2, 2048, 2048
H, HD = 16, 128
NCORES = 8
HPC = H // NCORES          # heads per core (2)
ROPE_BASE = 10000.0
SCALE = 1.0 / float(np.sqrt(HD))
SC = 512                   # QKV matmul free-dim chunk (s positions)
KSUB = D // 128            # 16 contraction subtiles
SCW = S // NCORES          # 256: per-core output cols per batch

_CACHE = {}


def _install_trace_shim():
    """Optionally register the axon NTFF profile hook (for test.py tracing)."""
    try:
        import types

        if "antenv.axon_hooks" in sys.modules:
            return True
        import antenv
        from trn_agent_boot.trn_boot import _ntff_profile_via_ctypes

        hook = _ntff_profile_via_ctypes("/opt/axon/libaxon_pjrt.so")
        mod = types.ModuleType("antenv.axon_hooks")
        _state = {"hook": hook}
        mod.get_axon_ntff_profile_hook = lambda: _state["hook"]
        mod.set_axon_ntff_profile_hook = lambda h: _state.__setitem__("hook", h)
        sys.modules["antenv.axon_hooks"] = mod
        antenv.axon_hooks = mod
        return True
    except Exception:
        return False


def _build():
    import concourse.bass as bass  # noqa: F401
    import concourse.mybir as mybir
    import concourse.tile as tile
    from concourse import bacc
    from concourse.masks import make_identity

    f32 = mybir.dt.float32
    f32r = mybir.dt.float32r
    EXP = mybir.ActivationFunctionType.Exp

    nc = bacc.Bacc("TRN2", target_bir_lowering=False, debug=False,
                   num_devices=NCORES)

    xT = nc.dram_tensor("xT", [128, KSUB, B * S], f32r, kind="ExternalInput")
    wqkv = nc.dram_tensor("wqkv", [128, KSUB, 3 * HPC * HD], f32r,
                          kind="ExternalInput")
    wout = nc.dram_tensor("wout", [128, KSUB, D], f32r, kind="ExternalInput")
    cosg = nc.dram_tensor("cosg", [128, S], f32, kind="ExternalInput")
    sing = nc.dram_tensor("sing", [128, S], f32, kind="ExternalInput")
    mneg = nc.dram_tensor("mneg", [128, 896], f32r, kind="ExternalInput")
    y = nc.dram_tensor("y", [B, SCW, D], f32, kind="ExternalOutput")

    NQC = S // SC          # qkv s-chunks per batch
    NKT = S // 128         # 16 key tiles
    VOFF = 2 * HPC * HD    # v block column offset in w_sb (512)

    with tile.TileContext(nc) as tc:
        with tc.tile_pool(name="const", bufs=1) as cp, \
             tc.tile_pool(name="stage", bufs=1) as stp, \
             tc.tile_pool(name="dram", bufs=1, space="DRAM") as dp, \
             tc.tile_pool(name="psA", bufs=4, space="PSUM") as psA, \
             tc.tile_pool(name="psOut", bufs=1, space="PSUM") as psO, \
             tc.tile_pool(name="w", bufs=1) as wp, \
             tc.tile_pool(name="xc", bufs=2) as xp, \
             tc.tile_pool(name="qkv", bufs=1) as qp, \
             tc.tile_pool(name="attn", bufs=1) as ap_, \
             tc.tile_pool(name="rotp", bufs=1) as rp, \
             tc.tile_pool(name="small", bufs=2) as ep:

            cos_sb = cp.tile([128, S], f32, name="cos_sb")
            sin_sb = cp.tile([128, S], f32, name="sin_sb")
            mneg_sb = cp.tile([128, 896], f32r, name="mneg_sb")
            ident = cp.tile([128, 128], f32, name="ident")
            identR = cp.tile([128, 128], f32r, name="identR")
            onesc = cp.tile([128, 1], f32, name="onesc")
            onescR = cp.tile([128, 1], f32r, name="onescR")
            onesr = cp.tile([1, 128], f32, name="onesr")
            onesrR = cp.tile([1, 128], f32r, name="onesrR")
            nc.sync.dma_start(cos_sb[:], cosg.ap())
            nc.sync.dma_start(sin_sb[:], sing.ap())
            nc.sync.dma_start(mneg_sb[:], mneg.ap())
            make_identity(nc, ident[:])
            nc.vector.tensor_copy(identR[:], ident[:])
            nc.vector.memset(onesc[:], 1.0)
            nc.vector.tensor_copy(onescR[:], onesc[:])
            nc.vector.memset(onesr[:], 1.0)
            nc.vector.tensor_copy(onesrR[:], onesr[:])

            ibs = [dp.tile([NCORES, HPC, 128, SCW], f32r, name=f"ib{b}")
                   for b in range(B)]
            obs = [dp.tile([NCORES, HPC, 128, SCW], f32r, name=f"ob{b}")
                   for b in range(B)]

            w_sb = wp.tile([128, KSUB, 3 * HPC * HD], f32r, tag="w_sb", name="w_sb")
            nc.sync.dma_start(w_sb[:], wqkv.ap())

            def qkv_rope(b):
                qkT = qp.tile([128, 2 * HPC, S], f32r, tag="qkT")
                Vn = qp.tile([128, NKT, HPC * HD], f32r, tag="Vn")
                for sc in range(NQC):
                    xc = xp.tile([128, KSUB, SC], f32r, tag="xc", name="xc")
                    off = b * S + sc * SC
                    nc.sync.dma_start(xc[:], xT.ap()[:, :, off:off + SC])
                    for m in range(2 * HPC):
                        ps = psA.tile([128, 512], f32, tag="bank")
                        for k in range(KSUB):
                            nc.tensor.matmul(
                                ps[:, :SC],
                                w_sb[:, k, m * 128:(m + 1) * 128],
                                xc[:, k],
                                start=(k == 0), stop=(k == KSUB - 1))
                        nc.vector.tensor_copy(
                            qkT[:, m, sc * SC:(sc + 1) * SC], ps[:, :SC])
                    for st2 in range(SC // 128):
                        ps = psA.tile([128, 512], f32, tag="bank")
                        for k in range(KSUB):
                            nc.tensor.matmul(
                                ps[:, :HPC * HD],
                                xc[:, k, st2 * 128:(st2 + 1) * 128],
                                w_sb[:, k, VOFF:VOFF + HPC * HD],
                                start=(k == 0), stop=(k == KSUB - 1))
                        nc.vector.tensor_copy(
                            Vn[:, sc * (SC // 128) + st2], ps[:, :HPC * HD])

                # RoPE, fused halves (sin grid stored pre-swapped):
                # rt[0:64] = t[64:128]*(-sin); rt[64:128] = t[0:64]*(+sin);
                # t *= cos; t += rt
                for m in range(2 * HPC):
                    rt = rp.tile([128, S], f32, tag="rot", name="rt")
                    nc.vector.tensor_mul(rt[0:64, :],
                                         qkT[64:128, m].bitcast(f32),
                                         sin_sb[64:128, :])
                    nc.vector.tensor_mul(rt[64:128, :],
                                         qkT[0:64, m].bitcast(f32),
                                         sin_sb[0:64, :])
                    nc.vector.tensor_mul(qkT[:, m], qkT[:, m], cos_sb[:])
                    nc.vector.tensor_add(qkT[:, m], qkT[:, m], rt[:])
                return qkT, Vn

            def attention(b, h, qkT, Vn):
                outT = psO.tile([128, S], f32, tag="outT")
                acc = ap_.tile([128, S], f32r, tag="acc")
                for kt in range(NKT):
                    q0 = 512 * (kt // 4)
                    nch = (S - q0) // 512
                    sps = []
                    for c in range(nch):
                        sp = psA.tile([128, 512], f32, tag="bank")
                        sps.append(sp)
                        if c == 0:
                            moff = 384 - 128 * (kt % 4)
                            nc.tensor.matmul(sp[:], identR[:],
                                             mneg_sb[:, moff:moff + 512],
                                             start=True, stop=False)
                    for c in range(nch):
                        qs = q0 + c * 512
                        nc.tensor.matmul(
                            sps[c][:],
                            qkT[:, HPC + h, kt * 128:(kt + 1) * 128],
                            qkT[:, h, qs:qs + 512],
                            start=(c != 0), stop=True)
                    ets = []
                    for c in range(nch):
                        et = ep.tile([128, 512], f32r, tag="expT")
                        ets.append(et)
                        nc.scalar.activation(et[:], sps[c][:], EXP, scale=SCALE)
                    for c in range(nch):
                        qs = q0 + c * 512
                        eng = nc.gpsimd if qs // 512 < 2 else nc.vector
                        if kt == 0:
                            eng.tensor_copy(acc[:, qs:qs + 512], ets[c][:])
                        else:
                            eng.tensor_add(acc[:, qs:qs + 512],
                                           acc[:, qs:qs + 512], ets[c][:])
                    for c in range(nch):
                        qs = q0 + c * 512
                        nc.tensor.matmul(
                            outT[:, qs:qs + 512],
                            Vn[:, kt, h * 128:(h + 1) * 128],
                            ets[c][:],
                            start=(kt == 0),
                            stop=(kt == 4 * (qs // 512) + 3))

                # denominators: partition-reduce via ones-matmul, reciprocal,
                # broadcast back via K=1 matmul; normalize out of PSUM.
                st = rp.tile([128, S], f32r, tag="rot", name="st")
                nc.vector.tensor_copy(st[:], outT[:])
                for j in range(4):
                    rps = psA.tile([128, 512], f32, tag="bank")
                    nc.tensor.matmul(rps[0:1, :], onescR[:],
                                     acc[:, j * 512:(j + 1) * 512],
                                     start=True, stop=True)
                    srow = stp.tile([1, 512], f32, tag="srow")
                    nc.vector.reciprocal_approx_fast(srow[:], rps[0:1, :])
                    srowR = stp.tile([1, 512], f32r, tag="srowR")
                    nc.vector.tensor_copy(srowR[:], srow[:])
                    bp = psA.tile([128, 512], f32, tag="bank")
                    nc.tensor.matmul(bp[:], onesrR[:], srowR[:],
                                     start=True, stop=True)
                    nc.vector.tensor_mul(st[:, j * 512:(j + 1) * 512],
                                         st[:, j * 512:(j + 1) * 512],
                                         bp[:])
                for j in range(NCORES):
                    nc.sync.dma_start(ibs[b][j, h],
                                      st[:, j * SCW:(j + 1) * SCW])

            def outproj(b):
                # reuses the w_sb slot (w_sb is dead after the last QKV)
                lhs = wp.tile([128, KSUB, SCW], f32r, tag="w_sb", name="lhs")
                nc.sync.dma_start(
                    lhs[:], obs[b][:].rearrange("i hh p s -> p (i hh) s"))
                for n in range(4):
                    # reuses the xc slots (QKV is done before any outproj)
                    wo = xp.tile([128, KSUB, 512], f32r, tag="xc", name="wo")
                    nc.sync.dma_start(wo[:],
                                      wout.ap()[:, :, n * 512:(n + 1) * 512])
                    for m in range(SCW // 128):
                        ps = psA.tile([128, 512], f32, tag="bank")
                        for k in range(KSUB):
                            nc.tensor.matmul(
                                ps[:],
                                lhs[:, k, m * 128:(m + 1) * 128],
                                wo[:, k],
                                start=(k == 0), stop=(k == KSUB - 1))
                        ys = stp.tile([128, 512], f32, tag="ys")
                        nc.vector.tensor_copy(ys[:], ps[:])
                        nc.sync.dma_start(
                            y.ap()[b, m * 128:(m + 1) * 128,
                                   n * 512:(n + 1) * 512],
                            ys[:])

            def a2a(b):
                nc.gpsimd.collective_compute(
                    "AllToAll", mybir.AluOpType.bypass,
                    replica_groups=[list(range(NCORES))],
                    ins=[ibs[b].opt()], outs=[obs[b].opt()])

            # batch 0 compute; its A2A runs while batch 1 computes;
            # outproj(0) slots into PE after batch 1's first head.
            qkT, Vn = qkv_rope(0)
            attention(0, 0, qkT, Vn)
            attention(0, 1, qkT, Vn)
            a2a(0)
            qkT, Vn = qkv_rope(1)
            attention(1, 0, qkT, Vn)
            outproj(0)
            attention(1, 1, qkT, Vn)
            a2a(1)
            outproj(1)

    nc.finalize()
    return nc


def _host_inputs(x, w_qkv, w_out):
    xTr = np.ascontiguousarray(
        x.reshape(B * S, D).T.reshape(KSUB, 128, B * S).transpose(1, 0, 2))
    woutr = np.ascontiguousarray(
        w_out.reshape(KSUB, 128, D).transpose(1, 0, 2))

    half = HD // 2
    inv = (1.0 / (ROPE_BASE ** (np.arange(half, dtype=np.float32) / half))
           ).astype(np.float32)
    ang = (np.arange(S, dtype=np.float32)[:, None] * inv[None, :])  # [S, 64]
    c = np.cos(ang).astype(np.float32).T      # [64, S]
    s = np.sin(ang).astype(np.float32).T
    cosg = np.ascontiguousarray(np.concatenate([c, c], axis=0))
    # pre-swapped: rows 0:64 = +sin (consumed against t[0:64] -> rt[64:128]),
    # rows 64:128 = -sin (consumed against t[64:128] -> rt[0:64])
    sing = np.ascontiguousarray(np.concatenate([s, -s], axis=0))

    # mneg strip: mneg[p, u] = 0 where (u-384) >= p else -1e9; the diagonal
    # mask for k-tile kt is the 512-wide slice at offset 384-128*(kt%4).
    u = np.arange(896)[None, :] - 384
    p = np.arange(128)[:, None]
    mneg = np.where(u >= p, 0.0, -1e9).astype(np.float32)

    maps = []
    for i in range(NCORES):
        h0, h1 = 2 * i, 2 * i + 1
        blocks = []
        for base in (0, D, 2 * D):
            blocks.append(w_qkv[:, base + 128 * h0:base + 128 * (h0 + 1)])
            blocks.append(w_qkv[:, base + 128 * h1:base + 128 * (h1 + 1)])
        shard = np.concatenate(blocks, axis=1)  # [D, 768]
        shard = np.ascontiguousarray(
            shard.reshape(KSUB, 128, 3 * HPC * HD).transpose(1, 0, 2))
        maps.append({"xT": xTr, "wqkv": shard, "wout": woutr,
                     "cosg": cosg, "sing": sing, "mneg": mneg})
    return maps


def kernel(x, w_qkv, w_out):
    from concourse.bass_utils import run_bass_kernel_spmd

    x = np.asarray(x, dtype=np.float32)
    w_qkv = np.asarray(w_qkv, dtype=np.float32)
    w_out = np.asarray(w_out, dtype=np.float32)

    if "nc" not in _CACHE:
        _CACHE["nc"] = _build()
    nc = _CACHE["nc"]

    trace = bool(int(os.environ.get("KERNEL_TRACE", "0")))
    if trace:
        trace = _install_trace_shim()

    in_maps = _host_inputs(x, w_qkv, w_out)
    res = run_bass_kernel_spmd(nc, in_maps, core_ids=list(range(NCORES)),
                               trace=trace)
    _CACHE["last_result"] = res
    # y per core i: [B, 256, D] = output rows [b*2048 + i*256, +256)
    full = np.empty((B * S, D), dtype=np.float32)
    for i in range(NCORES):
        yi = res.results[i]["y"]
        for b in range(B):
            full[b * S + i * SCW: b * S + (i + 1) * SCW] = yi[b]
    return full.reshape(B, S, D)


# revision 16
# speedup vs baseline: 1.4240x; 1.0251x over previous
"""Trainium2 Bass kernel for causal multi-head attention with RoPE.

Problem: x[2,2048,2048] -> qkv proj -> RoPE(q,k) -> causal softmax attention
(16 heads, hd=128) -> out proj.  Sharding: tensor-parallel over heads
(2 heads/core x 8 cores); the output projection contraction is restored
with one AllToAll per batch (head-shards -> sequence-shards), overlapped
with the other batch's compute, so each core computes a disjoint
[2, 256, 2048] slice of the final output.

All matmuls run as float32r (full-rate fp32 PE mode, ~1.6e-4 rel err on a
2048-deep contraction).  Softmax skips the max-subtraction (scores are
O(1) by construction); the causal mask is accumulated into PSUM as a
-1e9 constant via a PE identity-matmul; softmax denominators are
partition-reduced and broadcast back with tiny ones-matmuls on the PE.
"""

import os
import sys

if "/opt/trn_rl_repo" not in sys.path:
    sys.path.insert(0, "/opt/trn_rl_repo")

import numpy as np

B, S, D = 2, 2048, 2048
H, HD = 16, 128
NCORES = 8
HPC = H // NCORES          # heads per core (2)
ROPE_BASE = 10000.0
SCALE = 1.0 / float(np.sqrt(HD))
SC = 512                   # QKV matmul free-dim chunk (s positions)
KSUB = D // 128            # 16 contraction subtiles
SCW = S // NCORES          # 256: per-core output cols per batch

_CACHE = {}


def _install_trace_shim():
    """Optionally register the axon NTFF profile hook (for test.py tracing)."""
    try:
        import types

        if "antenv.axon_hooks" in sys.modules:
            return True
        import antenv
        from trn_agent_boot.trn_boot import _ntff_profile_via_ctypes

        hook = _ntff_profile_via_ctypes("/opt/axon/libaxon_pjrt.so")
        mod = types.ModuleType("antenv.axon_hooks")
        _state = {"hook": hook}
        mod.get_axon_ntff_profile_hook = lambda: _state["hook"]
        mod.set_axon_ntff_profile_hook = lambda h: _state.__setitem__("hook", h)
        sys.modules["antenv.axon_hooks"] = mod
        antenv.axon_hooks = mod
        return True
    except Exception:
        return False


def _build():
    import concourse.bass as bass  # noqa: F401
    import concourse.mybir as mybir
    import concourse.tile as tile
    from concourse import bacc
    from concourse.masks import make_identity

    f32 = mybir.dt.float32
    f32r = mybir.dt.float32r
    EXP = mybir.ActivationFunctionType.Exp

    nc = bacc.Bacc("TRN2", target_bir_lowering=False, debug=False,
                   num_devices=NCORES)

    xT = nc.dram_tensor("xT", [128, KSUB, B * S], f32r, kind="ExternalInput")
    wqkv = nc.dram_tensor("wqkv", [128, KSUB, 3 * HPC * HD], f32r,
                          kind="ExternalInput")
    wout = nc.dram_tensor("wout", [128, KSUB, D], f32r, kind="ExternalInput")
    cosg = nc.dram_tensor("cosg", [128, S], f32, kind="ExternalInput")
    sing = nc.dram_tensor("sing", [128, S], f32, kind="ExternalInput")
    mneg = nc.dram_tensor("mneg", [128, 896], f32r, kind="ExternalInput")
    y = nc.dram_tensor("y", [B, SCW, D], f32, kind="ExternalOutput")

    NQC = S // SC          # qkv s-chunks per batch
    NKT = S // 128         # 16 key tiles
    VOFF = 2 * HPC * HD    # v block column offset in w_sb (512)

    with tile.TileContext(nc) as tc:
        with tc.tile_pool(name="const", bufs=1) as cp, \
             tc.tile_pool(name="stage", bufs=1) as stp, \
             tc.tile_pool(name="dram", bufs=1, space="DRAM") as dp, \
             tc.tile_pool(name="psA", bufs=4, space="PSUM") as psA, \
             tc.tile_pool(name="psOut", bufs=1, space="PSUM") as psO, \
             tc.tile_pool(name="w", bufs=1) as wp, \
             tc.tile_pool(name="xc", bufs=2) as xp, \
             tc.tile_pool(name="qkv", bufs=1) as qp, \
             tc.tile_pool(name="attn", bufs=1) as ap_, \
             tc.tile_pool(name="rotp", bufs=1) as rp, \
             tc.tile_pool(name="small", bufs=3) as ep:

            cos_sb = cp.tile([128, S], f32, name="cos_sb")
            sin_sb = cp.tile([128, S], f32, name="sin_sb")
            mneg_sb = cp.tile([128, 896], f32r, name="mneg_sb")
            ident = cp.tile([128, 128], f32, name="ident")
            identR = cp.tile([128, 128], f32r, name="identR")
            onesc = cp.tile([128, 1], f32, name="onesc")
            onescR = cp.tile([128, 1], f32r, name="onescR")
            onesr = cp.tile([1, 128], f32, name="onesr")
            onesrR = cp.tile([1, 128], f32r, name="onesrR")
            nc.sync.dma_start(cos_sb[:], cosg.ap())
            nc.sync.dma_start(sin_sb[:], sing.ap())
            nc.sync.dma_start(mneg_sb[:], mneg.ap())
            make_identity(nc, ident[:])
            nc.vector.tensor_copy(identR[:], ident[:])
            nc.vector.memset(onesc[:], 1.0)
            nc.vector.tensor_copy(onescR[:], onesc[:])
            nc.vector.memset(onesr[:], 1.0)
            nc.vector.tensor_copy(onesrR[:], onesr[:])

            ibs = {(b, h): dp.tile([NCORES, 128, SCW], f32r, name=f"ib{b}{h}")
                   for b in range(B) for h in range(HPC)}
            obs = {(b, h): dp.tile([NCORES, 128, SCW], f32r, name=f"ob{b}{h}")
                   for b in range(B) for h in range(HPC)}

            w_sb = wp.tile([128, KSUB, 3 * HPC * HD], f32r, tag="w_sb", name="w_sb")
            nc.sync.dma_start(w_sb[:], wqkv.ap())

            def qkv_rope(b):
                qkT = qp.tile([128, 2 * HPC, S], f32r, tag="qkT")
                Vn = qp.tile([128, NKT, HPC * HD], f32r, tag="Vn")
                for sc in range(NQC):
                    xc = xp.tile([128, KSUB, SC], f32r, tag="xc", name="xc")
                    off = b * S + sc * SC
                    nc.sync.dma_start(xc[:], xT.ap()[:, :, off:off + SC])
                    for m in range(2 * HPC):
                        ps = psA.tile([128, 512], f32, tag="bank")
                        for k in range(KSUB):
                            nc.tensor.matmul(
                                ps[:, :SC],
                                w_sb[:, k, m * 128:(m + 1) * 128],
                                xc[:, k],
                                start=(k == 0), stop=(k == KSUB - 1))
                        nc.vector.tensor_copy(
                            qkT[:, m, sc * SC:(sc + 1) * SC], ps[:, :SC])
                    for st2 in range(SC // 128):
                        ps = psA.tile([128, 512], f32, tag="bank")
                        for k in range(KSUB):
                            nc.tensor.matmul(
                                ps[:, :HPC * HD],
                                xc[:, k, st2 * 128:(st2 + 1) * 128],
                                w_sb[:, k, VOFF:VOFF + HPC * HD],
                                start=(k == 0), stop=(k == KSUB - 1))
                        nc.vector.tensor_copy(
                            Vn[:, sc * (SC // 128) + st2], ps[:, :HPC * HD])

                # RoPE, fused halves (sin grid stored pre-swapped):
                # rt[0:64] = t[64:128]*(-sin); rt[64:128] = t[0:64]*(+sin);
                # t *= cos; t += rt
                for m in range(2 * HPC):
                    rt = rp.tile([128, S], f32, tag="rot", name="rt")
                    nc.vector.tensor_mul(rt[0:64, :],
                                         qkT[64:128, m].bitcast(f32),
                                         sin_sb[64:128, :])
                    nc.vector.tensor_mul(rt[64:128, :],
                                         qkT[0:64, m].bitcast(f32),
                                         sin_sb[0:64, :])
                    nc.vector.tensor_mul(qkT[:, m], qkT[:, m], cos_sb[:])
                    nc.vector.tensor_add(qkT[:, m], qkT[:, m], rt[:])
                return qkT, Vn

            def attention(b, h, qkT, Vn):
                outT = psO.tile([128, S], f32, tag="outT")
                acc = ap_.tile([128, S], f32r, tag="acc")

                def emit_av(kt, ets):
                    q0 = 512 * (kt // 4)
                    for c in range(len(ets)):
                        qs = q0 + c * 512
                        nc.tensor.matmul(
                            outT[:, qs:qs + 512],
                            Vn[:, kt, h * 128:(h + 1) * 128],
                            ets[c][:],
                            start=(kt == 0),
                            stop=(kt == 4 * (qs // 512) + 3))

                prev = None
                for kt in range(NKT):
                    q0 = 512 * (kt // 4)
                    nch = (S - q0) // 512
                    sps = []
                    for c in range(nch):
                        sp = psA.tile([128, 512], f32, tag="bank")
                        sps.append(sp)
                        if c == 0:
                            moff = 384 - 128 * (kt % 4)
                            nc.tensor.matmul(sp[:], identR[:],
                                             mneg_sb[:, moff:moff + 512],
                                             start=True, stop=False)
                    for c in range(nch):
                        qs = q0 + c * 512
                        nc.tensor.matmul(
                            sps[c][:],
                            qkT[:, HPC + h, kt * 128:(kt + 1) * 128],
                            qkT[:, h, qs:qs + 512],
                            start=(c != 0), stop=True)
                    if prev is not None:
                        emit_av(*prev)
                    ets = []
                    for c in range(nch):
                        et = ep.tile([128, 512], f32r, tag="expT")
                        ets.append(et)
                        nc.scalar.activation(et[:], sps[c][:], EXP, scale=SCALE)
                    for c in range(nch):
                        qs = q0 + c * 512
                        if kt == 0:
                            nc.vector.tensor_copy(acc[:, qs:qs + 512], ets[c][:])
                        else:
                            eng = nc.gpsimd if qs // 512 < 2 else nc.vector
                            eng.tensor_add(acc[:, qs:qs + 512],
                                           acc[:, qs:qs + 512], ets[c][:])
                    prev = (kt, ets)
                emit_av(*prev)

                # denominators: partition-reduce via ones-matmul, reciprocal,
                # broadcast back via K=1 matmul; normalize out of PSUM.
                st = rp.tile([128, S], f32r, tag="rot", name="st")
                nc.vector.tensor_copy(st[:], outT[:])
                for j in range(4):
                    rps = psA.tile([128, 512], f32, tag="bank")
                    nc.tensor.matmul(rps[0:1, :], onescR[:],
                                     acc[:, j * 512:(j + 1) * 512],
                                     start=True, stop=True)
                    srow = stp.tile([1, 512], f32, tag="srow")
                    nc.vector.reciprocal_approx_fast(srow[:], rps[0:1, :])
                    bp = psA.tile([128, 512], f32, tag="bank")
                    nc.tensor.matmul(bp[:], onesr[:], srow[:],
                                     start=True, stop=True)
                    nc.vector.tensor_mul(st[:, j * 512:(j + 1) * 512],
                                         st[:, j * 512:(j + 1) * 512],
                                         bp[:])
                for j in range(NCORES):
                    nc.sync.dma_start(ibs[(b, h)][j],
                                      st[:, j * SCW:(j + 1) * SCW])

            def outproj(b):
                # reuses the w_sb slot (w_sb is dead after the last QKV)
                # k-subtile order: hh*8 + i  <->  global head 2i+hh (wout is
                # permuted host-side to match).
                lhs = wp.tile([128, KSUB, SCW], f32r, tag="w_sb", name="lhs")
                for hh in range(HPC):
                    nc.sync.dma_start(
                        lhs[:, hh * NCORES:(hh + 1) * NCORES, :],
                        obs[(b, hh)][:].rearrange("i p s -> p i s"))
                for n in range(4):
                    # reuses the xc slots (QKV is done before any outproj)
                    wo = xp.tile([128, KSUB, 512], f32r, tag="xc", name="wo")
                    nc.sync.dma_start(wo[:],
                                      wout.ap()[:, :, n * 512:(n + 1) * 512])
                    for m in range(SCW // 128):
                        ps = psA.tile([128, 512], f32, tag="bank")
                        for k in range(KSUB):
                            nc.tensor.matmul(
                                ps[:],
                                lhs[:, k, m * 128:(m + 1) * 128],
                                wo[:, k],
                                start=(k == 0), stop=(k == KSUB - 1))
                        ys = stp.tile([128, 512], f32, tag="ys")
                        nc.vector.tensor_copy(ys[:], ps[:])
                        nc.sync.dma_start(
                            y.ap()[b, m * 128:(m + 1) * 128,
                                   n * 512:(n + 1) * 512],
                            ys[:])

            def a2a(b, h):
                nc.gpsimd.collective_compute(
                    "AllToAll", mybir.AluOpType.bypass,
                    replica_groups=[list(range(NCORES))],
                    ins=[ibs[(b, h)].opt()], outs=[obs[(b, h)].opt()])

            # batch 0 compute; its A2A runs while batch 1 computes;
            # outproj(0) slots into PE after batch 1's first head.
            qkT, Vn = qkv_rope(0)
            attention(0, 0, qkT, Vn)
            a2a(0, 0)
            attention(0, 1, qkT, Vn)
            a2a(0, 1)
            qkT, Vn = qkv_rope(1)
            attention(1, 0, qkT, Vn)
            a2a(1, 0)
            outproj(0)
            attention(1, 1, qkT, Vn)
            a2a(1, 1)
            outproj(1)

    nc.finalize()
    return nc


def _host_inputs(x, w_qkv, w_out):
    xTr = np.ascontiguousarray(
        x.reshape(B * S, D).T.reshape(KSUB, 128, B * S).transpose(1, 0, 2))
    horder = [2 * i + hh for hh in range(HPC) for i in range(NCORES)]
    woutr = np.ascontiguousarray(
        w_out.reshape(H, HD, D)[horder].transpose(1, 0, 2))

    half = HD // 2
    inv = (1.0 / (ROPE_BASE ** (np.arange(half, dtype=np.float32) / half))
           ).astype(np.float32)
    ang = (np.arange(S, dtype=np.float32)[:, None] * inv[None, :])  # [S, 64]
    c = np.cos(ang).astype(np.float32).T      # [64, S]
    s = np.sin(ang).astype(np.float32).T
    cosg = np.ascontiguousarray(np.concatenate([c, c], axis=0))
    # pre-swapped: rows 0:64 = +sin (consumed against t[0:64] -> rt[64:128]),
    # rows 64:128 = -sin (consumed against t[64:128] -> rt[0:64])
    sing = np.ascontiguousarray(np.concatenate([s, -s], axis=0))

    # mneg strip: mneg[p, u] = 0 where (u-384) >= p else -1e9; the diagonal
    # mask for k-tile kt is the 512-wide slice at offset 384-128*(kt%4).
    u = np.arange(896)[None, :] - 384
    p = np.arange(128)[:, None]
    mneg = np.where(u >= p, 0.0, -1e9).astype(np.float32)

    maps = []
    for i in range(NCORES):
        h0, h1 = 2 * i, 2 * i + 1
        blocks = []
        for base in (0, D, 2 * D):
            blocks.append(w_qkv[:, base + 128 * h0:base + 128 * (h0 + 1)])
            blocks.append(w_qkv[:, base + 128 * h1:base + 128 * (h1 + 1)])
        shard = np.concatenate(blocks, axis=1)  # [D, 768]
        shard = np.ascontiguousarray(
            shard.reshape(KSUB, 128, 3 * HPC * HD).transpose(1, 0, 2))
        maps.append({"xT": xTr, "wqkv": shard, "wout": woutr,
                     "cosg": cosg, "sing": sing, "mneg": mneg})
    return maps


def kernel(x, w_qkv, w_out):
    from concourse.bass_utils import run_bass_kernel_spmd

    x = np.asarray(x, dtype=np.float32)
    w_qkv = np.asarray(w_qkv, dtype=np.float32)
    w_out = np.asarray(w_out, dtype=np.float32)

    if "nc" not in _CACHE:
        _CACHE["nc"] = _build()
    nc = _CACHE["nc"]

    trace = bool(int(os.environ.get("KERNEL_TRACE", "0")))
    if trace:
        trace = _install_trace_shim()

    in_maps = _host_inputs(x, w_qkv, w_out)
    res = run_bass_kernel_spmd(nc, in_maps, core_ids=list(range(NCORES)),
                               trace=trace)
    _CACHE["last_result"] = res
    # y per core i: [B, 256, D] = output rows [b*2048 + i*256, +256)
    full = np.empty((B * S, D), dtype=np.float32)
    for i in range(NCORES):
        yi = res.results[i]["y"]
        for b in range(B):
            full[b * S + i * SCW: b * S + (i + 1) * SCW] = yi[b]
    return full.reshape(B, S, D)


# revision 19
# speedup vs baseline: 1.5147x; 1.0637x over previous
"""Trainium2 Bass kernel for causal multi-head attention with RoPE.

Problem: x[2,2048,2048] -> qkv proj -> RoPE(q,k) -> causal softmax attention
(16 heads, hd=128) -> out proj.  Sharding: tensor-parallel over heads
(2 heads/core x 8 cores); the output projection contraction is restored
with one AllToAll per batch (head-shards -> sequence-shards), overlapped
with the other batch's compute, so each core computes a disjoint
[2, 256, 2048] slice of the final output.

All matmuls run as float32r (full-rate fp32 PE mode, ~1.6e-4 rel err on a
2048-deep contraction).  Softmax skips the max-subtraction (scores are
O(1) by construction); the causal mask is accumulated into PSUM as a
-1e9 constant via a PE identity-matmul; softmax denominators are
partition-reduced and broadcast back with tiny ones-matmuls on the PE.
"""

import os
import sys

if "/opt/trn_rl_repo" not in sys.path:
    sys.path.insert(0, "/opt/trn_rl_repo")

import numpy as np

B, S, D = 2, 2048, 2048
H, HD = 16, 128
NCORES = 8
HPC = H // NCORES          # heads per core (2)
ROPE_BASE = 10000.0
SCALE = 1.0 / float(np.sqrt(HD))
SC = 512                   # QKV matmul free-dim chunk (s positions)
KSUB = D // 128            # 16 contraction subtiles
SCW = S // NCORES          # 256: per-core output cols per batch

_CACHE = {}


def _install_trace_shim():
    """Optionally register the axon NTFF profile hook (for test.py tracing)."""
    try:
        import types

        if "antenv.axon_hooks" in sys.modules:
            return True
        import antenv
        from trn_agent_boot.trn_boot import _ntff_profile_via_ctypes

        hook = _ntff_profile_via_ctypes("/opt/axon/libaxon_pjrt.so")
        mod = types.ModuleType("antenv.axon_hooks")
        _state = {"hook": hook}
        mod.get_axon_ntff_profile_hook = lambda: _state["hook"]
        mod.set_axon_ntff_profile_hook = lambda h: _state.__setitem__("hook", h)
        sys.modules["antenv.axon_hooks"] = mod
        antenv.axon_hooks = mod
        return True
    except Exception:
        return False


def _build():
    import concourse.bass as bass  # noqa: F401
    import concourse.mybir as mybir
    import concourse.tile as tile
    from concourse import bacc
    from concourse.masks import make_identity

    f32 = mybir.dt.float32
    f32r = mybir.dt.float32r
    EXP = mybir.ActivationFunctionType.Exp

    nc = bacc.Bacc("TRN2", target_bir_lowering=False, debug=False,
                   num_devices=NCORES)

    xT = nc.dram_tensor("xT", [128, KSUB, B * S], f32r, kind="ExternalInput")
    wqkv = nc.dram_tensor("wqkv", [128, KSUB, 3 * HPC * HD], f32r,
                          kind="ExternalInput")
    wout = nc.dram_tensor("wout", [128, KSUB, D], f32r, kind="ExternalInput")
    cosg = nc.dram_tensor("cosg", [128, S], f32, kind="ExternalInput")
    sing = nc.dram_tensor("sing", [128, S], f32, kind="ExternalInput")
    mneg = nc.dram_tensor("mneg", [128, 896], f32r, kind="ExternalInput")
    y = nc.dram_tensor("y", [B, SCW, D], f32, kind="ExternalOutput")

    NQC = S // SC          # qkv s-chunks per batch
    NKT = S // 128         # 16 key tiles
    VOFF = 2 * HPC * HD    # v block column offset in w_sb (512)

    with tile.TileContext(nc) as tc:
        with tc.tile_pool(name="const", bufs=1) as cp, \
             tc.tile_pool(name="stage", bufs=1) as stp, \
             tc.tile_pool(name="dram", bufs=1, space="DRAM") as dp, \
             tc.tile_pool(name="psA", bufs=4, space="PSUM") as psA, \
             tc.tile_pool(name="psOut", bufs=1, space="PSUM") as psO, \
             tc.tile_pool(name="w", bufs=1) as wp, \
             tc.tile_pool(name="xc", bufs=2) as xp, \
             tc.tile_pool(name="qkv", bufs=1) as qp, \
             tc.tile_pool(name="attn", bufs=1) as ap_, \
             tc.tile_pool(name="rotp", bufs=1) as rp, \
             tc.tile_pool(name="small", bufs=3) as ep:

            cos_sb = cp.tile([128, S], f32, name="cos_sb")
            sin_sb = cp.tile([128, S], f32, name="sin_sb")
            mneg_sb = cp.tile([128, 896], f32r, name="mneg_sb")
            ident = cp.tile([128, 128], f32, name="ident")
            identR = cp.tile([128, 128], f32r, name="identR")
            onesc = cp.tile([128, 1], f32, name="onesc")
            onescR = cp.tile([128, 1], f32r, name="onescR")
            onesr = cp.tile([1, 128], f32, name="onesr")
            onesrR = cp.tile([1, 128], f32r, name="onesrR")
            nc.sync.dma_start(cos_sb[:], cosg.ap())
            nc.sync.dma_start(sin_sb[:], sing.ap())
            nc.sync.dma_start(mneg_sb[:], mneg.ap())
            make_identity(nc, ident[:])
            nc.vector.tensor_copy(identR[:], ident[:])
            nc.vector.memset(onesc[:], 1.0)
            nc.vector.tensor_copy(onescR[:], onesc[:])
            nc.vector.memset(onesr[:], 1.0)
            nc.vector.tensor_copy(onesrR[:], onesr[:])

            ibs = {(b, h): dp.tile([NCORES, 128, SCW], f32r, name=f"ib{b}{h}")
                   for b in range(B) for h in range(HPC)}
            obs = {(b, h): dp.tile([NCORES, 128, SCW], f32r, name=f"ob{b}{h}")
                   for b in range(B) for h in range(HPC)}

            wqk_t = []
            for m in range(2 * HPC):
                wt = wp.tile([128, KSUB, 128], f32r, tag=f"w{m}", name=f"w{m}")
                nc.sync.dma_start(wt[:],
                                  wqkv.ap()[:, :, m * 128:(m + 1) * 128])
                wqk_t.append(wt)
            wv_t = wp.tile([128, KSUB, HPC * HD], f32r, tag="wv", name="wv")
            nc.sync.dma_start(wv_t[:], wqkv.ap()[:, :, VOFF:VOFF + HPC * HD])

            def qkv_rope(b):
                qkT = qp.tile([128, 2 * HPC, S], f32r, tag="qkT")
                Vn = qp.tile([128, NKT, HPC * HD], f32r, tag="Vn")
                for sc in range(NQC):
                    xc = xp.tile([128, KSUB, SC], f32r, tag="xc", name="xc")
                    off = b * S + sc * SC
                    nc.sync.dma_start(xc[:], xT.ap()[:, :, off:off + SC])
                    for m in range(2 * HPC):
                        ps = psA.tile([128, 512], f32, tag="bank")
                        for k in range(KSUB):
                            nc.tensor.matmul(
                                ps[:, :SC],
                                wqk_t[m][:, k],
                                xc[:, k],
                                start=(k == 0), stop=(k == KSUB - 1))
                        nc.vector.tensor_copy(
                            qkT[:, m, sc * SC:(sc + 1) * SC], ps[:, :SC])
                    for st2 in range(SC // 128):
                        ps = psA.tile([128, 512], f32, tag="bank")
                        for k in range(KSUB):
                            nc.tensor.matmul(
                                ps[:, :HPC * HD],
                                xc[:, k, st2 * 128:(st2 + 1) * 128],
                                wv_t[:, k],
                                start=(k == 0), stop=(k == KSUB - 1))
                        nc.vector.tensor_copy(
                            Vn[:, sc * (SC // 128) + st2], ps[:, :HPC * HD])

                # RoPE, fused halves (sin grid stored pre-swapped):
                # rt[0:64] = t[64:128]*(-sin); rt[64:128] = t[0:64]*(+sin);
                # t *= cos; t += rt
                for m in range(2 * HPC):
                    rt = rp.tile([128, S], f32, tag="rot", name="rt")
                    nc.vector.tensor_mul(rt[0:64, :],
                                         qkT[64:128, m].bitcast(f32),
                                         sin_sb[64:128, :])
                    nc.vector.tensor_mul(rt[64:128, :],
                                         qkT[0:64, m].bitcast(f32),
                                         sin_sb[0:64, :])
                    nc.vector.tensor_mul(qkT[:, m], qkT[:, m], cos_sb[:])
                    nc.vector.tensor_add(qkT[:, m], qkT[:, m], rt[:])
                return qkT, Vn

            def attention(b, h, qkT, Vn):
                outT = psO.tile([128, S], f32, tag="outT")
                acc = ap_.tile([128, S], f32r, tag="acc")

                def emit_av(kt, ets):
                    q0 = 512 * (kt // 4)
                    for c in range(len(ets)):
                        qs = q0 + c * 512
                        nc.tensor.matmul(
                            outT[:, qs:qs + 512],
                            Vn[:, kt, h * 128:(h + 1) * 128],
                            ets[c][:],
                            start=(kt == 0),
                            stop=(kt == 4 * (qs // 512) + 3))

                prev = None
                for kt in range(NKT):
                    q0 = 512 * (kt // 4)
                    nch = (S - q0) // 512
                    sps = []
                    for c in range(nch):
                        sp = psA.tile([128, 512], f32, tag="bank")
                        sps.append(sp)
                        if c == 0:
                            moff = 384 - 128 * (kt % 4)
                            nc.tensor.matmul(sp[:], identR[:],
                                             mneg_sb[:, moff:moff + 512],
                                             start=True, stop=False)
                    for c in range(nch):
                        qs = q0 + c * 512
                        nc.tensor.matmul(
                            sps[c][:],
                            qkT[:, HPC + h, kt * 128:(kt + 1) * 128],
                            qkT[:, h, qs:qs + 512],
                            start=(c != 0), stop=True)
                    if prev is not None:
                        emit_av(*prev)
                    ets = []
                    for c in range(nch):
                        et = ep.tile([128, 512], f32r, tag="expT")
                        ets.append(et)
                        nc.scalar.activation(et[:], sps[c][:], EXP, scale=SCALE)
                    for c in range(nch):
                        qs = q0 + c * 512
                        if kt == 0:
                            nc.vector.tensor_copy(acc[:, qs:qs + 512], ets[c][:])
                        else:
                            eng = nc.gpsimd if qs // 512 < 2 else nc.vector
                            eng.tensor_add(acc[:, qs:qs + 512],
                                           acc[:, qs:qs + 512], ets[c][:])
                    prev = (kt, ets)
                emit_av(*prev)

                # denominators: partition-reduce via ones-matmul, reciprocal,
                # broadcast back via K=1 matmul; normalize out of PSUM.
                st = rp.tile([128, S], f32r, tag="rot", name="st")
                nc.vector.tensor_copy(st[:], outT[:])
                for j in range(4):
                    rps = psA.tile([128, 512], f32, tag="bank")
                    nc.tensor.matmul(rps[0:1, :], onescR[:],
                                     acc[:, j * 512:(j + 1) * 512],
                                     start=True, stop=True)
                    srow = stp.tile([1, 512], f32, tag="srow")
                    nc.vector.reciprocal_approx_fast(srow[:], rps[0:1, :])
                    bp = psA.tile([128, 512], f32, tag="bank")
                    nc.tensor.matmul(bp[:], onesr[:], srow[:],
                                     start=True, stop=True)
                    nc.vector.tensor_mul(st[:, j * 512:(j + 1) * 512],
                                         st[:, j * 512:(j + 1) * 512],
                                         bp[:])
                for j in range(NCORES):
                    nc.sync.dma_start(ibs[(b, h)][j],
                                      st[:, j * SCW:(j + 1) * SCW])

            def outproj(b):
                # reuses the w_sb slot (w_sb is dead after the last QKV)
                # k-subtile order: hh*8 + i  <->  global head 2i+hh (wout is
                # permuted host-side to match).
                lhs = wp.tile([128, KSUB, SCW], f32r, tag="wv", name="lhs")
                for hh in range(HPC):
                    nc.sync.dma_start(
                        lhs[:, hh * NCORES:(hh + 1) * NCORES, :],
                        obs[(b, hh)][:].rearrange("i p s -> p i s"))
                for n in range(4):
                    # reuses the xc slots (QKV is done before any outproj)
                    wo = xp.tile([128, KSUB, 512], f32r, tag="xc", name="wo")
                    nc.sync.dma_start(wo[:],
                                      wout.ap()[:, :, n * 512:(n + 1) * 512])
                    for m in range(SCW // 128):
                        ps = psA.tile([128, 512], f32, tag="bank")
                        for k in range(KSUB):
                            nc.tensor.matmul(
                                ps[:],
                                lhs[:, k, m * 128:(m + 1) * 128],
                                wo[:, k],
                                start=(k == 0), stop=(k == KSUB - 1))
                        ys = stp.tile([128, 512], f32, tag="ys")
                        nc.vector.tensor_copy(ys[:], ps[:])
                        nc.sync.dma_start(
                            y.ap()[b, m * 128:(m + 1) * 128,
                                   n * 512:(n + 1) * 512],
                            ys[:])

            def a2a(b, h):
                nc.gpsimd.collective_compute(
                    "AllToAll", mybir.AluOpType.bypass,
                    replica_groups=[list(range(NCORES))],
                    ins=[ibs[(b, h)].opt()], outs=[obs[(b, h)].opt()])

            # batch 0 compute; its A2A runs while batch 1 computes;
            # outproj(0) slots into PE after batch 1's first head.
            qkT, Vn = qkv_rope(0)
            attention(0, 0, qkT, Vn)
            a2a(0, 0)
            attention(0, 1, qkT, Vn)
            a2a(0, 1)
            qkT, Vn = qkv_rope(1)
            attention(1, 0, qkT, Vn)
            a2a(1, 0)
            attention(1, 1, qkT, Vn)
            a2a(1, 1)
            outproj(0)
            outproj(1)

    nc.finalize()
    return nc


def _host_inputs(x, w_qkv, w_out):
    xTr = np.ascontiguousarray(
        x.reshape(B * S, D).T.reshape(KSUB, 128, B * S).transpose(1, 0, 2))
    horder = [2 * i + hh for hh in range(HPC) for i in range(NCORES)]
    woutr = np.ascontiguousarray(
        w_out.reshape(H, HD, D)[horder].transpose(1, 0, 2))

    half = HD // 2
    inv = (1.0 / (ROPE_BASE ** (np.arange(half, dtype=np.float32) / half))
           ).astype(np.float32)
    ang = (np.arange(S, dtype=np.float32)[:, None] * inv[None, :])  # [S, 64]
    c = np.cos(ang).astype(np.float32).T      # [64, S]
    s = np.sin(ang).astype(np.float32).T
    cosg = np.ascontiguousarray(np.concatenate([c, c], axis=0))
    # pre-swapped: rows 0:64 = +sin (consumed against t[0:64] -> rt[64:128]),
    # rows 64:128 = -sin (consumed against t[64:128] -> rt[0:64])
    sing = np.ascontiguousarray(np.concatenate([s, -s], axis=0))

    # mneg strip: mneg[p, u] = 0 where (u-384) >= p else -1e9; the diagonal
    # mask for k-tile kt is the 512-wide slice at offset 384-128*(kt%4).
    u = np.arange(896)[None, :] - 384
    p = np.arange(128)[:, None]
    mneg = np.where(u >= p, 0.0, -1e9).astype(np.float32)

    maps = []
    for i in range(NCORES):
        h0, h1 = 2 * i, 2 * i + 1
        blocks = []
        for base in (0, D, 2 * D):
            blocks.append(w_qkv[:, base + 128 * h0:base + 128 * (h0 + 1)])
            blocks.append(w_qkv[:, base + 128 * h1:base + 128 * (h1 + 1)])
        shard = np.concatenate(blocks, axis=1)  # [D, 768]
        shard = np.ascontiguousarray(
            shard.reshape(KSUB, 128, 3 * HPC * HD).transpose(1, 0, 2))
        maps.append({"xT": xTr, "wqkv": shard, "wout": woutr,
                     "cosg": cosg, "sing": sing, "mneg": mneg})
    return maps


def kernel(x, w_qkv, w_out):
    from concourse.bass_utils import run_bass_kernel_spmd

    x = np.asarray(x, dtype=np.float32)
    w_qkv = np.asarray(w_qkv, dtype=np.float32)
    w_out = np.asarray(w_out, dtype=np.float32)

    if "nc" not in _CACHE:
        _CACHE["nc"] = _build()
    nc = _CACHE["nc"]

    trace = bool(int(os.environ.get("KERNEL_TRACE", "0")))
    if trace:
        trace = _install_trace_shim()

    in_maps = _host_inputs(x, w_qkv, w_out)
    res = run_bass_kernel_spmd(nc, in_maps, core_ids=list(range(NCORES)),
                               trace=trace)
    _CACHE["last_result"] = res
    # y per core i: [B, 256, D] = output rows [b*2048 + i*256, +256)
    full = np.empty((B * S, D), dtype=np.float32)
    for i in range(NCORES):
        yi = res.results[i]["y"]
        for b in range(B):
            full[b * S + i * SCW: b * S + (i + 1) * SCW] = yi[b]
    return full.reshape(B, S, D)


# revision 21
# speedup vs baseline: 1.5838x; 1.0456x over previous
"""Trainium2 Bass kernel for causal multi-head attention with RoPE.

Problem: x[2,2048,2048] -> qkv proj -> RoPE(q,k) -> causal softmax attention
(16 heads, hd=128) -> out proj.  Sharding: tensor-parallel over heads
(2 heads/core x 8 cores); the output projection contraction is restored
with one AllToAll per batch (head-shards -> sequence-shards), overlapped
with the other batch's compute, so each core computes a disjoint
[2, 256, 2048] slice of the final output.

All matmuls run as float32r (full-rate fp32 PE mode, ~1.6e-4 rel err on a
2048-deep contraction).  Softmax skips the max-subtraction (scores are
O(1) by construction); the causal mask is accumulated into PSUM as a
-1e9 constant via a PE identity-matmul; softmax denominators are
partition-reduced and broadcast back with tiny ones-matmuls on the PE.
"""

import os
import sys

if "/opt/trn_rl_repo" not in sys.path:
    sys.path.insert(0, "/opt/trn_rl_repo")

import numpy as np

B, S, D = 2, 2048, 2048
H, HD = 16, 128
NCORES = 8
HPC = H // NCORES          # heads per core (2)
ROPE_BASE = 10000.0
SCALE = 1.0 / float(np.sqrt(HD))
SC = 512                   # QKV matmul free-dim chunk (s positions)
KSUB = D // 128            # 16 contraction subtiles
SCW = S // NCORES          # 256: per-core output cols per batch

_CACHE = {}


def _install_trace_shim():
    """Optionally register the axon NTFF profile hook (for test.py tracing)."""
    try:
        import types

        if "antenv.axon_hooks" in sys.modules:
            return True
        import antenv
        from trn_agent_boot.trn_boot import _ntff_profile_via_ctypes

        hook = _ntff_profile_via_ctypes("/opt/axon/libaxon_pjrt.so")
        mod = types.ModuleType("antenv.axon_hooks")
        _state = {"hook": hook}
        mod.get_axon_ntff_profile_hook = lambda: _state["hook"]
        mod.set_axon_ntff_profile_hook = lambda h: _state.__setitem__("hook", h)
        sys.modules["antenv.axon_hooks"] = mod
        antenv.axon_hooks = mod
        return True
    except Exception:
        return False


def _build():
    import concourse.bass as bass  # noqa: F401
    import concourse.mybir as mybir
    import concourse.tile as tile
    from concourse import bacc
    from concourse.masks import make_identity

    f32 = mybir.dt.float32
    f32r = mybir.dt.float32r
    EXP = mybir.ActivationFunctionType.Exp

    nc = bacc.Bacc("TRN2", target_bir_lowering=False, debug=False,
                   num_devices=NCORES)

    xT = nc.dram_tensor("xT", [128, KSUB, B * S], f32r, kind="ExternalInput")
    wqkv = nc.dram_tensor("wqkv", [128, KSUB, 3 * HPC * HD], f32r,
                          kind="ExternalInput")
    wout = nc.dram_tensor("wout", [128, KSUB, D], f32r, kind="ExternalInput")
    cosg = nc.dram_tensor("cosg", [128, S], f32, kind="ExternalInput")
    sing = nc.dram_tensor("sing", [128, S], f32, kind="ExternalInput")
    mneg = nc.dram_tensor("mneg", [128, 896], f32r, kind="ExternalInput")
    y = nc.dram_tensor("y", [B, SCW, D], f32, kind="ExternalOutput")

    NQC = S // SC          # qkv s-chunks per batch
    NKT = S // 128         # 16 key tiles
    VOFF = 2 * HPC * HD    # v block column offset in w_sb (512)

    with tile.TileContext(nc) as tc:
        with tc.tile_pool(name="const", bufs=1) as cp, \
             tc.tile_pool(name="stage", bufs=1) as stp, \
             tc.tile_pool(name="dram", bufs=1, space="DRAM") as dp, \
             tc.tile_pool(name="psA", bufs=4, space="PSUM") as psA, \
             tc.tile_pool(name="psOut", bufs=1, space="PSUM") as psO, \
             tc.tile_pool(name="w", bufs=1) as wp, \
             tc.tile_pool(name="xc", bufs=2) as xp, \
             tc.tile_pool(name="qkv", bufs=1) as qp, \
             tc.tile_pool(name="attn", bufs=1) as ap_, \
             tc.tile_pool(name="rotp", bufs=1) as rp, \
             tc.tile_pool(name="small", bufs=3) as ep:

            cos_sb = cp.tile([128, S], f32, name="cos_sb")
            sin_sb = cp.tile([128, S], f32, name="sin_sb")
            mneg_sb = cp.tile([128, 896], f32r, name="mneg_sb")
            ident = cp.tile([128, 128], f32, name="ident")
            identR = cp.tile([128, 128], f32r, name="identR")
            onesc = cp.tile([128, 1], f32, name="onesc")
            onescR = cp.tile([128, 1], f32r, name="onescR")
            onesr = cp.tile([1, 128], f32, name="onesr")
            onesrR = cp.tile([1, 128], f32r, name="onesrR")
            nc.sync.dma_start(cos_sb[:], cosg.ap())
            nc.sync.dma_start(sin_sb[:], sing.ap())
            nc.sync.dma_start(mneg_sb[:], mneg.ap())
            make_identity(nc, ident[:])
            nc.vector.tensor_copy(identR[:], ident[:])
            nc.vector.memset(onesc[:], 1.0)
            nc.vector.tensor_copy(onescR[:], onesc[:])
            nc.vector.memset(onesr[:], 1.0)
            nc.vector.tensor_copy(onesrR[:], onesr[:])

            ibs = {(b, h): dp.tile([NCORES, 128, SCW], f32r, name=f"ib{b}{h}")
                   for b in range(B) for h in range(HPC)}
            obs = {(b, h): dp.tile([NCORES, 128, SCW], f32r, name=f"ob{b}{h}")
                   for b in range(B) for h in range(HPC)}

            wqk_t = []
            for m in range(2 * HPC):
                wt = wp.tile([128, KSUB, 128], f32r, tag=f"w{m}", name=f"w{m}")
                nc.sync.dma_start(wt[:],
                                  wqkv.ap()[:, :, m * 128:(m + 1) * 128])
                wqk_t.append(wt)
            wv_t = wp.tile([128, KSUB, HPC * HD], f32r, tag="wv", name="wv")
            nc.sync.dma_start(wv_t[:], wqkv.ap()[:, :, VOFF:VOFF + HPC * HD])

            def qkv_rope(b):
                qkT = qp.tile([128, 2 * HPC, S], f32r, tag="qkT")
                Vn = qp.tile([128, NKT, HPC * HD], f32r, tag="Vn")
                for sc in range(NQC):
                    xc = xp.tile([128, KSUB, SC], f32r, tag="xc", name="xc")
                    off = b * S + sc * SC
                    nc.sync.dma_start(xc[:], xT.ap()[:, :, off:off + SC])
                    for m in range(2 * HPC):
                        ps = psA.tile([128, 512], f32, tag="bank")
                        for k in range(KSUB):
                            nc.tensor.matmul(
                                ps[:, :SC],
                                wqk_t[m][:, k],
                                xc[:, k],
                                start=(k == 0), stop=(k == KSUB - 1))
                        nc.vector.tensor_copy(
                            qkT[:, m, sc * SC:(sc + 1) * SC], ps[:, :SC])
                    for st2 in range(SC // 128):
                        ps = psA.tile([128, 512], f32, tag="bank")
                        for k in range(KSUB):
                            nc.tensor.matmul(
                                ps[:, :HPC * HD],
                                xc[:, k, st2 * 128:(st2 + 1) * 128],
                                wv_t[:, k],
                                start=(k == 0), stop=(k == KSUB - 1))
                        nc.vector.tensor_copy(
                            Vn[:, sc * (SC // 128) + st2], ps[:, :HPC * HD])

                # RoPE, fused halves (sin grid stored pre-swapped):
                # rt[0:64] = t[64:128]*(-sin); rt[64:128] = t[0:64]*(+sin);
                # t *= cos; t += rt
                for m in range(2 * HPC):
                    rt = rp.tile([128, S], f32, tag="rot", name="rt")
                    nc.vector.tensor_mul(rt[0:64, :],
                                         qkT[64:128, m].bitcast(f32),
                                         sin_sb[64:128, :])
                    nc.vector.tensor_mul(rt[64:128, :],
                                         qkT[0:64, m].bitcast(f32),
                                         sin_sb[0:64, :])
                    nc.vector.tensor_mul(qkT[:, m], qkT[:, m], cos_sb[:])
                    nc.vector.tensor_add(qkT[:, m], qkT[:, m], rt[:])
                return qkT, Vn

            def attention(b, h, qkT, Vn):
                outT = psO.tile([128, S], f32, tag="outT")
                acc = ap_.tile([128, S], f32r, tag="acc")

                def emit_av(kt, off, ets):
                    q0 = 512 * (kt // 4)
                    for c in range(len(ets)):
                        qs = q0 + c * 512
                        o = off if c == 0 else 0
                        nc.tensor.matmul(
                            outT[:, qs + o:qs + 512],
                            Vn[:, kt, h * 128:(h + 1) * 128],
                            ets[c][:, o:512],
                            start=(kt == 0),
                            stop=(kt == 4 * (qs // 512) + 3))

                prev = None
                for kt in range(NKT):
                    q0 = 512 * (kt // 4)
                    off = 128 * (kt % 4)   # causal start within chunk 0
                    nch = (S - q0) // 512
                    sps = []
                    for c in range(nch):
                        sp = psA.tile([128, 512], f32, tag="bank")
                        sps.append(sp)
                        if c == 0:
                            # -1e9 upper-tri mask for the diagonal 128 block
                            nc.tensor.matmul(sp[:, off:512], identR[:],
                                             mneg_sb[:, 384:896 - off],
                                             start=True, stop=False)
                    for c in range(nch):
                        qs = q0 + c * 512
                        o = off if c == 0 else 0
                        nc.tensor.matmul(
                            sps[c][:, o:512],
                            qkT[:, HPC + h, kt * 128:(kt + 1) * 128],
                            qkT[:, h, qs + o:qs + 512],
                            start=(c != 0), stop=True)
                    if prev is not None:
                        emit_av(*prev)
                    ets = []
                    for c in range(nch):
                        o = off if c == 0 else 0
                        et = ep.tile([128, 512], f32r, tag="expT")
                        ets.append(et)
                        nc.scalar.activation(et[:, o:512], sps[c][:, o:512],
                                             EXP, scale=SCALE)
                    for c in range(nch):
                        qs = q0 + c * 512
                        o = off if c == 0 else 0
                        if kt == 0:
                            nc.vector.tensor_copy(acc[:, qs:qs + 512], ets[c][:])
                        else:
                            eng = nc.gpsimd if qs // 512 < 2 else nc.vector
                            eng.tensor_add(acc[:, qs + o:qs + 512],
                                           acc[:, qs + o:qs + 512],
                                           ets[c][:, o:512])
                    prev = (kt, off, ets)
                emit_av(*prev)

                # denominators: partition-reduce via ones-matmul, reciprocal,
                # broadcast back via K=1 matmul; normalize out of PSUM.
                st = rp.tile([128, S], f32r, tag="rot", name="st")
                nc.vector.tensor_copy(st[:], outT[:])
                for j in range(4):
                    rps = psA.tile([128, 512], f32, tag="bank")
                    nc.tensor.matmul(rps[0:1, :], onescR[:],
                                     acc[:, j * 512:(j + 1) * 512],
                                     start=True, stop=True)
                    srow = stp.tile([1, 512], f32, tag="srow")
                    nc.vector.reciprocal_approx_fast(srow[:], rps[0:1, :])
                    bp = psA.tile([128, 512], f32, tag="bank")
                    nc.tensor.matmul(bp[:], onesr[:], srow[:],
                                     start=True, stop=True)
                    nc.vector.tensor_mul(st[:, j * 512:(j + 1) * 512],
                                         st[:, j * 512:(j + 1) * 512],
                                         bp[:])
                for j in range(NCORES):
                    nc.sync.dma_start(ibs[(b, h)][j],
                                      st[:, j * SCW:(j + 1) * SCW])

            def outproj(b):
                # reuses the w_sb slot (w_sb is dead after the last QKV)
                # k-subtile order: hh*8 + i  <->  global head 2i+hh (wout is
                # permuted host-side to match).
                lhs = wp.tile([128, KSUB, SCW], f32r, tag="wv", name="lhs")
                for hh in range(HPC):
                    nc.sync.dma_start(
                        lhs[:, hh * NCORES:(hh + 1) * NCORES, :],
                        obs[(b, hh)][:].rearrange("i p s -> p i s"))
                for n in range(4):
                    # reuses the xc slots (QKV is done before any outproj)
                    wo = xp.tile([128, KSUB, 512], f32r, tag="xc", name="wo")
                    nc.sync.dma_start(wo[:],
                                      wout.ap()[:, :, n * 512:(n + 1) * 512])
                    for m in range(SCW // 128):
                        ps = psA.tile([128, 512], f32, tag="bank")
                        for k in range(KSUB):
                            nc.tensor.matmul(
                                ps[:],
                                lhs[:, k, m * 128:(m + 1) * 128],
                                wo[:, k],
                                start=(k == 0), stop=(k == KSUB - 1))
                        ys = stp.tile([128, 512], f32, tag="ys")
                        nc.vector.tensor_copy(ys[:], ps[:])
                        nc.sync.dma_start(
                            y.ap()[b, m * 128:(m + 1) * 128,
                                   n * 512:(n + 1) * 512],
                            ys[:])

            def a2a(b, h):
                nc.gpsimd.collective_compute(
                    "AllToAll", mybir.AluOpType.bypass,
                    replica_groups=[list(range(NCORES))],
                    ins=[ibs[(b, h)].opt()], outs=[obs[(b, h)].opt()])

            # batch 0 compute; its A2A runs while batch 1 computes;
            # outproj(0) slots into PE after batch 1's first head.
            qkT, Vn = qkv_rope(0)
            attention(0, 0, qkT, Vn)
            a2a(0, 0)
            attention(0, 1, qkT, Vn)
            a2a(0, 1)
            qkT, Vn = qkv_rope(1)
            attention(1, 0, qkT, Vn)
            a2a(1, 0)
            attention(1, 1, qkT, Vn)
            a2a(1, 1)
            outproj(0)
            outproj(1)

    nc.finalize()
    return nc


def _host_inputs(x, w_qkv, w_out):
    xTr = np.ascontiguousarray(
        x.reshape(B * S, D).T.reshape(KSUB, 128, B * S).transpose(1, 0, 2))
    horder = [2 * i + hh for hh in range(HPC) for i in range(NCORES)]
    woutr = np.ascontiguousarray(
        w_out.reshape(H, HD, D)[horder].transpose(1, 0, 2))

    half = HD // 2
    inv = (1.0 / (ROPE_BASE ** (np.arange(half, dtype=np.float32) / half))
           ).astype(np.float32)
    ang = (np.arange(S, dtype=np.float32)[:, None] * inv[None, :])  # [S, 64]
    c = np.cos(ang).astype(np.float32).T      # [64, S]
    s = np.sin(ang).astype(np.float32).T
    cosg = np.ascontiguousarray(np.concatenate([c, c], axis=0))
    # pre-swapped: rows 0:64 = +sin (consumed against t[0:64] -> rt[64:128]),
    # rows 64:128 = -sin (consumed against t[64:128] -> rt[0:64])
    sing = np.ascontiguousarray(np.concatenate([s, -s], axis=0))

    # mneg strip: mneg[p, u] = 0 where (u-384) >= p else -1e9; the diagonal
    # mask for k-tile kt is the 512-wide slice at offset 384-128*(kt%4).
    u = np.arange(896)[None, :] - 384
    p = np.arange(128)[:, None]
    mneg = np.where(u >= p, 0.0, -1e9).astype(np.float32)

    maps = []
    for i in range(NCORES):
        h0, h1 = 2 * i, 2 * i + 1
        blocks = []
        for base in (0, D, 2 * D):
            blocks.append(w_qkv[:, base + 128 * h0:base + 128 * (h0 + 1)])
            blocks.append(w_qkv[:, base + 128 * h1:base + 128 * (h1 + 1)])
        shard = np.concatenate(blocks, axis=1)  # [D, 768]
        shard = np.ascontiguousarray(
            shard.reshape(KSUB, 128, 3 * HPC * HD).transpose(1, 0, 2))
        maps.append({"xT": xTr, "wqkv": shard, "wout": woutr,
                     "cosg": cosg, "sing": sing, "mneg": mneg})
    return maps


def kernel(x, w_qkv, w_out):
    from concourse.bass_utils import run_bass_kernel_spmd

    x = np.asarray(x, dtype=np.float32)
    w_qkv = np.asarray(w_qkv, dtype=np.float32)
    w_out = np.asarray(w_out, dtype=np.float32)

    if "nc" not in _CACHE:
        _CACHE["nc"] = _build()
    nc = _CACHE["nc"]

    trace = bool(int(os.environ.get("KERNEL_TRACE", "0")))
    if trace:
        trace = _install_trace_shim()

    in_maps = _host_inputs(x, w_qkv, w_out)
    res = run_bass_kernel_spmd(nc, in_maps, core_ids=list(range(NCORES)),
                               trace=trace)
    _CACHE["last_result"] = res
    # y per core i: [B, 256, D] = output rows [b*2048 + i*256, +256)
    full = np.empty((B * S, D), dtype=np.float32)
    for i in range(NCORES):
        yi = res.results[i]["y"]
        for b in range(B):
            full[b * S + i * SCW: b * S + (i + 1) * SCW] = yi[b]
    return full.reshape(B, S, D)
